# revision 47
# baseline (speedup 1.0000x reference)
"""Trainium2 Bass kernel for nn_Block_56616258896419 (moe_routing).

Self-contained: takes FULL inputs (as from setup_inputs()), returns FULL
[4,1024,1024] f32 output. Internally shards across 8 NeuronCores:
  - tokens 8-way (core r: batch r//2, sequence half r%2) for attention/LN
  - experts 8-way (core r computes expert r over ALL tokens) for the MoE
Collectives: pairwise AllGather of K/V, 8-way AllGather of LN1'd
activations (transposed, bf16), 4x chunked 8-way ReduceScatter of the
prob-weighted expert outputs (overlapped with MoE compute).
"""
import numpy as np
import ml_dtypes

B, S, E, H, HD, NEXP, FF = 4, 1024, 1024, 16, 64, 8, 4096
NCORE = 8
TOK = 512          # tokens per core
TC = 256           # MoE token-chunk
NCHUNK = (B * S) // TC
EPS = 1e-5
BF16 = ml_dtypes.bfloat16
F8 = ml_dtypes.float8_e4m3

_CACHE = {}


def _build_program():
    import concourse.bacc as bacc
    import concourse.mybir as mybir
    import concourse.tile as tile

    dt = mybir.dt
    f32, bf = dt.float32, dt.bfloat16
    AF = mybir.ActivationFunctionType
    ALU = mybir.AluOpType

    nc = bacc.Bacc("TRN2", target_bir_lowering=False, debug=False,
                   num_devices=NCORE)

    # ---------------- I/O ----------------
    def inp(name, shape, d):
        return nc.dram_tensor(name, shape, d, kind="ExternalInput").ap()

    fp8i = dt.float8e4
    xT_d = inp("xT", [128, 2 * 4096], fp8i)       # x^T [own|partner], e-tiled
    xr_d = inp("xr", [128, 4 * 1024], f32)        # x token-major, tt blocks
    xr2_d = inp("xr2", [64, 8 * 1024], f32)       # x rows per src-core group
    wqkv_d = inp("wqkv", [128, 8 * 3072], fp8i)   # [E,3E] e-tiled, x64
    bqk_d = inp("bqk", [128, 16], f32)            # x64
    bv_d = inp("bv", [1, 1024], bf)               # x64
    wp_d = inp("wp", [128, 8 * 1024], fp8i)       # x64
    bp_d = inp("bp", [1, 1024], bf)               # x64
    masks_d = inp("masks", [4, 128, 256], bf)     # diag-pair 0/1 masks
    ident_d = inp("ident", [128, 128], bf)
    fp8 = dt.float8e4
    w1_d = inp("w1", [128, 8 * 4096], fp8)        # ln1-folded, x16, DR pairs
    b1_d = inp("b1", [128, 32], f32)              # x16
    w2_d = inp("w2", [128, 32 * 1024], fp8)       # x64, DR pairs
    b2_d = inp("b2", [1, 1024], bf)               # x1024
    wr_d = inp("wr", [128, 8 * 8], fp8)           # ln1-folded, permuted, x64
    br_d = inp("br", [1, 8], bf)
    g2_d = inp("g2", [128, 1024], f32)            # ln2_g replicated
    bl2_d = inp("bl2", [128, 1024], f32)          # ln2_b replicated
    out_d = nc.dram_tensor("out", [8, 64, 1024], f32,
                           kind="ExternalOutput").ap()

    # ---------------- internal DRAM ----------------
    hag_inA1 = nc.dram_tensor("hag_inA1", [8, 128, 128], fp8).ap()
    hag_inA2 = nc.dram_tensor("hag_inA2", [8, 128, 128], fp8).ap()
    hag_inB = nc.dram_tensor("hag_inB", [8, 128, 256], fp8).ap()
    hag_outA1 = nc.dram_tensor("hag_outA1", [8, 8, 128, 128], fp8,
                               addr_space="Shared").ap()
    hag_outA2 = nc.dram_tensor("hag_outA2", [8, 8, 128, 128], fp8,
                               addr_space="Shared").ap()
    hag_outB = nc.dram_tensor("hag_outB", [8, 8, 128, 256], fp8,
                              addr_space="Shared").ap()
    rs_ins = [nc.dram_tensor(f"rs_in{g}", [512, 1024], bf).ap()
              for g in range(8)]
    rs_outs = [nc.dram_tensor(f"rs_out{g}", [64, 1024], bf).ap()
               for g in range(8)]

    with tile.TileContext(nc) as tc:
        cpool_cm = tc.tile_pool(name="cpool", bufs=1, side="left")
        cpool = cpool_cm.__enter__()
        ones_row = cpool.tile([1, 128], bf)
        nc.vector.memset(ones_row[:], 1.0)
        ones_f = cpool.tile([1, 128], f32)
        nc.vector.memset(ones_f[:], 1.0)
        bqk_sb = cpool.tile([128, 16], f32)
        nc.sync.dma_start(bqk_sb[:], bqk_d[:])
        bv_sb = cpool.tile([1, 1024], bf)
        nc.sync.dma_start(bv_sb[:], bv_d[:])
        bp_sb = cpool.tile([1, 1024], bf)
        nc.sync.dma_start(bp_sb[:], bp_d[:])
        ident_sb = cpool.tile([128, 128], bf)
        nc.sync.dma_start(ident_sb[:], ident_d[:])
        wr_sb = cpool.tile([128, 64], fp8)
        nc.sync.dma_start(wr_sb[:], wr_d[:])
        br_sb = cpool.tile([1, 8], bf)
        nc.sync.dma_start(br_sb[:], br_d[:])
        b1_sb = cpool.tile([128, 32], f32)
        nc.sync.dma_start(b1_sb[:], b1_d[:])
        b2_sb = cpool.tile([1, 1024], bf)
        nc.sync.dma_start(b2_sb[:], b2_d[:])

        # ===== phase 1: local K/V for BOTH interleave-halves, then Q =====
        # all fp8 DoubleRow over e-tile pairs; weights x64, descaled on exit
        DR = mybir.MatmulPerfMode.DoubleRow
        qkv_cm = tc.tile_pool(name="qkv", bufs=1, side="right")
        qkv = qkv_cm.__enter__()
        xT_sb = qkv.tile([128, 8192], fp8)
        nc.sync.dma_start(xT_sb[:, 0:4096], xT_d[:, 0:4096])
        wqkv_sb = qkv.tile([128, 24576], fp8)
        # sect-major host layout [k|v|q]: contiguous section DMAs, k first
        nc.sync.dma_start(wqkv_sb[:, 0:8192], wqkv_d[:, 0:8192])
        nc.sync.dma_start(xT_sb[:, 4096:8192], xT_d[:, 4096:8192])
        nc.sync.dma_start(wqkv_sb[:, 16384:24576], wqkv_d[:, 16384:24576])
        nc.sync.dma_start(wqkv_sb[:, 8192:16384], wqkv_d[:, 8192:16384])
        xT_v = xT_sb[:].rearrange("p (h e t) -> p h e t", h=2, e=8)
        wq_v = wqkv_sb[:].rearrange("p (s e c) -> p s e c", s=3, e=8)

        attn_cm = tc.tile_pool(name="attn", bufs=1, side="left")
        attn = attn_cm.__enter__()
        qT_sb = attn.tile([128, 4096], bf)
        kT_full = attn.tile([128, 8192], bf)     # [j][half*512 + s]
        v_full = attn.tile([128, 8192], bf)      # [u = half*4+tt][hd]
        # augmented V: per t-tile, 16 heads x (64 v-cols + 1 ones-col);
        # ones-cols set up front, v copied per u-tile inside the V loop
        v_aug = attn.tile([128, 8 * 1040], bf)
        for tt in range(8):
            nc.vector.memset(
                v_aug[:, tt * 1040: tt * 1040 + 1040]
                .rearrange("p (h dd) -> p h dd", dd=65)[:, :, 64:65], 1.0)

        with tc.tile_pool(name="ps_qkv", bufs=3, space="PSUM") as psq:
            for half in range(2):
                for j in range(8):
                    k_ps = psq.tile([128, 512], f32, tag="qk_ps")
                    for p in range(4):
                        nc.tensor.matmul(
                            k_ps[:],
                            wq_v[:, 0, 2 * p: 2 * p + 2,
                                 j * 128: j * 128 + 128],
                            xT_v[:, half, 2 * p: 2 * p + 2, :],
                            start=(p == 0), stop=(p == 3), perf_mode=DR)
                    nc.vector.tensor_scalar(
                        kT_full[:, j * 1024 + half * 512:
                                j * 1024 + half * 512 + 512], k_ps[:],
                        bqk_sb[:, 8 + j: 8 + j + 1], 1.0 / 64.0,
                        op0=ALU.add, op1=ALU.mult)
            # qT next (own tokens = half 0) so scores can start right away
            for j in range(8):
                q_ps = psq.tile([128, 512], f32, tag="qk_ps")
                for p in range(4):
                    nc.tensor.matmul(
                        q_ps[:],
                        wq_v[:, 2, 2 * p: 2 * p + 2, j * 128: j * 128 + 128],
                        xT_v[:, 0, 2 * p: 2 * p + 2, :],
                        start=(p == 0), stop=(p == 3), perf_mode=DR)
                nc.vector.tensor_scalar(
                    qT_sb[:, j * 512: j * 512 + 512], q_ps[:],
                    bqk_sb[:, j: j + 1], 0.125 / 64.0,
                    op0=ALU.add, op1=ALU.mult)
            for half in range(2):
                for tt in range(4):
                    u = half * 4 + tt
                    for c in range(2):
                        v_ps = psq.tile([128, 512], f32, tag="v_ps")
                        for p in range(4):
                            nc.tensor.matmul(
                                v_ps[:],
                                xT_v[:, half, 2 * p: 2 * p + 2,
                                     tt * 128: tt * 128 + 128],
                                wq_v[:, 1, 2 * p: 2 * p + 2,
                                     c * 512: c * 512 + 512],
                                start=(p == 0), stop=False, perf_mode=DR)
                        nc.tensor.matmul(
                            v_ps[:], ones_row[:, 0:128],
                            bv_sb[:, c * 512: c * 512 + 512],
                            start=False, stop=True)
                        nc.scalar.activation(
                            v_full[:, u * 1024 + c * 512:
                                   u * 1024 + c * 512 + 512], v_ps[:],
                            AF.Identity, scale=1.0 / 64.0)
                    nc.vector.tensor_copy(
                        v_aug[:, u * 1040: u * 1040 + 1040]
                        .rearrange("p (h dd) -> p h dd", dd=65)[:, :, 0:64],
                        v_full[:, u * 1024: u * 1024 + 1024]
                        .rearrange("p (h dd) -> p h dd", dd=64))
        qkv_cm.__exit__(None, None, None)

        # MoE weights prefetch on the vector ring (overlaps attention);
        # wp/x residual loads early on the sync ring so proj never waits
        moe_cm = tc.tile_pool(name="moe", bufs=1, side="right")
        moe = moe_cm.__enter__()
        w1_sb = moe.tile([128, 32768], fp8)
        w2_sb = moe.tile([128, 32768], fp8)
        for et in range(8):
            nc.scalar.dma_start(w1_sb[:, et * 4096: et * 4096 + 4096],
                                w1_d[:, et * 4096: et * 4096 + 4096])
        for ft8 in range(8):
            nc.scalar.dma_start(w2_sb[:, ft8 * 4096: ft8 * 4096 + 4096],
                                w2_d[:, ft8 * 4096: ft8 * 4096 + 4096])

        # ============ phase 2: attention ============
        mask_sb = attn.tile([128, 1024], bf)
        for ss in range(4):
            nc.sync.dma_start(mask_sb[:, ss * 256: ss * 256 + 256],
                              masks_d[ss])

        proj_cm = tc.tile_pool(name="proj", bufs=1, side="right")
        projp = proj_cm.__enter__()
        catT_sb = projp.tile([128, 4096], bf)    # unnormalized heads^T
        catT8 = projp.tile([128, 4096], fp8)     # normalized, fp8 for proj
        wp_sb = projp.tile([128, 8192], fp8)
        nc.sync.dma_start(wp_sb[:], wp_d[:])
        x_sb = projp.tile([128, 4096], f32)
        nc.sync.dma_start(x_sb[:], xr_d[:])

        with tc.tile_pool(name="sc", bufs=4, side="left") as scp, \
             tc.tile_pool(name="ps_sc", bufs=3, space="PSUM") as ps_sc, \
             tc.tile_pool(name="ps_av", bufs=4, space="PSUM") as ps_av:
            # one N-restricted matmul per key-tile: keys (ph,tl) attend to
            # queries >= tl, so scores/exp/AV each run [128, (4-tl)*128].
            # PE emission is software-pipelined: scores run one key-tile
            # ahead of AV (covering exp latency), and each j's epilogue
            # (softmax-normalize via K=1 matmuls) is deferred into j+1.

            def epilogue(j, avps):
                rcs = []
                for h01 in range(2):
                    nc.scalar.copy(
                        catT_sb[64 * h01:64 * h01 + 64,
                                j * 512: j * 512 + 512],
                        avps[h01][0:64, :])
                    sm_f = scp.tile([1, 512], f32, tag="sm_f", bufs=4,
                                    name=f"smf{j}_{h01}")
                    nc.scalar.copy(sm_f[:], avps[h01][64:65, :])
                    rc_f = scp.tile([1, 512], f32, tag="rc_f", bufs=4,
                                    name=f"rcf{j}_{h01}")
                    nc.vector.reciprocal_approx_fast(rc_f[:], sm_f[:])
                    rc_h = scp.tile([1, 512], bf, tag="rc_h", bufs=4,
                                    name=f"rc{j}_{h01}")
                    nc.vector.tensor_copy(rc_h[:], rc_f[:])
                    rcs.append(rc_h)
                bc_ps = ps_sc.tile([128, 512], f32, tag="sc",
                                   name=f"bc{j}")[:, 0:512]
                nc.tensor.matmul(bc_ps[0:64, :], ones_row[:, 0:64],
                                 rcs[0][:], start=True, stop=True)
                nc.tensor.matmul(bc_ps[64:128, :], ones_row[:, 0:64],
                                 rcs[1][:], start=True, stop=True)
                nc.vector.tensor_tensor(
                    catT_sb[:, j * 512: j * 512 + 512],
                    catT_sb[:, j * 512: j * 512 + 512], bc_ps[:],
                    op=ALU.mult)
                nc.scalar.copy(catT8[:, j * 512: j * 512 + 512],
                               catT_sb[:, j * 512: j * 512 + 512])

            pend = None
            for j in range(8):
                avps = [ps_av.tile([65, 512], f32, tag="av", name=f"av{j}_{k}")
                        for k in range(2)]
                for h01 in range(2):
                    po = 64 * h01
                    prev = None
                    first = True
                    for ph in range(2):
                        for tl in range(4):
                            N = (4 - tl) * 128
                            qo = tl * 128
                            sc_ps = ps_sc.tile([128, 512], f32, tag="sc",
                                               name=f"sc{j}_{h01}_{ph}_{tl}")
                            nc.tensor.matmul(
                                sc_ps[:, 0:N],
                                kT_full[po:po + 64,
                                        j * 1024 + ph * 512 + tl * 128:
                                        j * 1024 + ph * 512 + tl * 128 + 128],
                                qT_sb[po:po + 64,
                                      j * 512 + qo: j * 512 + 512],
                                start=True, stop=True)
                            expT = scp.tile([128, 512], bf, tag="expT",
                                            name=f"ex{j}_{h01}_{ph}_{tl}")
                            nc.scalar.activation(
                                expT[:, 0:N], sc_ps[:, 0:N], AF.Exp)
                            nc.vector.tensor_tensor(
                                expT[:, 0:128], expT[:, 0:128],
                                mask_sb[:, tl * 256 + ph * 128:
                                        tl * 256 + ph * 128 + 128],
                                op=ALU.mult)
                            if prev is not None:
                                pexpT, pN, pqo, pu = prev
                                nc.tensor.matmul(
                                    avps[h01][:, pqo:512],
                                    v_aug[:, pu * 1040 + (2 * j + h01) * 65:
                                          pu * 1040 + (2 * j + h01) * 65 + 65],
                                    pexpT[:, 0:pN],
                                    start=first, stop=False)
                                first = False
                            if pend is not None and h01 == 0 and \
                                    ph == 1 and tl == 1:
                                epilogue(*pend)
                                pend = None
                            prev = (expT, N, qo, ph * 4 + tl)
                    pexpT, pN, pqo, pu = prev
                    nc.tensor.matmul(
                        avps[h01][:, pqo:512],
                        v_aug[:, pu * 1040 + (2 * j + h01) * 65:
                              pu * 1040 + (2 * j + h01) * 65 + 65],
                        pexpT[:, 0:pN],
                        start=first, stop=True)
                pend = (j, avps)
            epilogue(*pend)
        attn_cm.__exit__(None, None, None)

        h_sb = projp.tile([128, 4096], bf)
        hT_stage = projp.tile([128, 4096], fp8)
        cat_v = catT8[:].rearrange("p (j t) -> p j t", j=8)
        wp_v = wp_sb[:].rearrange("p (j e) -> p j e", j=8)

        with tc.tile_pool(name="prw", bufs=2, side="left") as prp, \
             tc.tile_pool(name="ps_pr", bufs=4, space="PSUM") as ps_pr:
            for tt in range(4):
                y_sb = prp.tile([128, 1024], f32, tag="y")
                for ec in range(2):
                    ao_ps = ps_pr.tile([128, 512], f32, tag="ao")
                    for p in range(4):
                        nc.tensor.matmul(
                            ao_ps[:],
                            cat_v[:, 2 * p: 2 * p + 2,
                                  tt * 128: tt * 128 + 128],
                            wp_v[:, 2 * p: 2 * p + 2,
                                 ec * 512: ec * 512 + 512],
                            start=(p == 0), stop=False, perf_mode=DR)
                    nc.tensor.matmul(
                        ao_ps[:], ones_row[:, 0:128],
                        bp_sb[:, ec * 512: ec * 512 + 512],
                        start=False, stop=True)
                    nc.vector.tensor_scalar(
                        y_sb[:, ec * 512: ec * 512 + 512], ao_ps[:],
                        1.0 / 64.0, None, op0=ALU.mult)
                    nc.vector.tensor_tensor(
                        y_sb[:, ec * 512: ec * 512 + 512],
                        y_sb[:, ec * 512: ec * 512 + 512],
                        x_sb[:, tt * 1024 + ec * 512: tt * 1024 + ec * 512 + 512],
                        op=ALU.add)
                # LN1 stats
                mean = prp.tile([128, 1], f32, tag="mean")
                nc.vector.reduce_sum(mean[:], y_sb[:], axis=mybir.AxisListType.X)
                nc.vector.tensor_scalar_mul(mean[:], mean[:], 1.0 / 1024.0)
                sq = prp.tile([128, 1024], bf, tag="sq")
                sqs = prp.tile([128, 1], f32, tag="sqs")
                nc.scalar.activation(sq[:], y_sb[:], AF.Square,
                                     accum_out=sqs[:])
                m2 = prp.tile([128, 1], f32, tag="m2")
                nc.scalar.activation(m2[:], mean[:], AF.Square)
                var = prp.tile([128, 1], f32, tag="var")
                nc.vector.tensor_scalar(var[:], sqs[:], 1.0 / 1024.0, EPS,
                                        op0=ALU.mult, op1=ALU.add)
                nc.vector.tensor_tensor(var[:], var[:], m2[:], op=ALU.subtract)
                std = prp.tile([128, 1], f32, tag="std")
                nc.scalar.activation(std[:], var[:], AF.Sqrt)
                rstd = prp.tile([128, 1], f32, tag="rstd")
                nc.vector.reciprocal(rstd[:], std[:])
                nc.vector.tensor_scalar(
                    h_sb[:, tt * 1024: tt * 1024 + 1024], y_sb[:],
                    mean[:], rstd[:], op0=ALU.subtract, op1=ALU.mult)
                # transpose h tile -> hT
                for et in range(8):
                    tp = ps_pr.tile([128, 128], bf, tag="tp")
                    nc.tensor.transpose(
                        tp[:], h_sb[:, tt * 1024 + et * 128:
                                    tt * 1024 + et * 128 + 128], ident_sb[:])
                    nc.scalar.copy(
                        hT_stage[:, et * 512 + tt * 128:
                                 et * 512 + tt * 128 + 128], tp[:])
                if tt == 0:
                    for et in range(8):
                        nc.gpsimd.dma_start(
                            hag_inA1[et],
                            hT_stage[:, et * 512: et * 512 + 128])
                    nc.gpsimd.collective_compute(
                        "AllGather", mybir.AluOpType.bypass,
                        replica_groups=[list(range(8))],
                        ins=[hag_inA1.opt()], outs=[hag_outA1.opt()])
                if tt == 1:
                    for et in range(8):
                        nc.gpsimd.dma_start(
                            hag_inA2[et],
                            hT_stage[:, et * 512 + 128: et * 512 + 256])
                    nc.gpsimd.collective_compute(
                        "AllGather", mybir.AluOpType.bypass,
                        replica_groups=[list(range(8))],
                        ins=[hag_inA2.opt()], outs=[hag_outA2.opt()])
                if tt == 3:
                    for et in range(8):
                        nc.gpsimd.dma_start(
                            hag_inB[et],
                            hT_stage[:, et * 512 + 256: et * 512 + 512])
                    nc.gpsimd.collective_compute(
                        "AllGather", mybir.AluOpType.bypass,
                        replica_groups=[list(range(8))],
                        ins=[hag_inB.opt()], outs=[hag_outB.opt()])
            # (b) PE warm-keepers over the h-AG wait
            for wi in range(16):
                wm_ps = ps_pr.tile([128, 512], f32, tag="ao", name=f"wm{wi}")
                nc.tensor.matmul(wm_ps[:], catT8[:, 0:128],
                                 wp_sb[:, 0:512], start=True, stop=True)
        proj_cm.__exit__(None, None, None)

        # ============ phase 3: MoE (expert r over all tokens) ============
        # fin (residual+LN2) work is interleaved per completed RS group so
        # only the last group's LN2 sits on the tail
        with tc.tile_pool(name="fin", bufs=2, side="left") as fin, \
             tc.tile_pool(name="mchunk", bufs=2, side="left") as mck, \
             tc.tile_pool(name="ps_md", bufs=2, space="PSUM") as ps_md, \
             tc.tile_pool(name="ps_eo", bufs=3, space="PSUM") as ps_eo:
            x2_sb = fin.tile([64, 8192], f32, bufs=1)
            nc.sync.dma_start(x2_sb[:], xr2_d[:])
            g2_sb = fin.tile([128, 1024], f32, bufs=1)
            nc.sync.dma_start(g2_sb[:], g2_d[:])
            bl2_sb = fin.tile([128, 1024], f32, bufs=1)
            nc.sync.dma_start(bl2_sb[:], bl2_d[:])

            def do_fin(g):
                y2b = fin.tile([64, 1024], bf, tag="y2b", name=f"y2b{g}")
                nc.sync.dma_start(y2b[:], rs_outs[g][:])
                y2 = fin.tile([64, 1024], f32, tag="y2", name=f"y2{g}")
                nc.vector.tensor_tensor(
                    y2[:], y2b[:], x2_sb[:, g * 1024: g * 1024 + 1024],
                    op=ALU.add)
                mean = fin.tile([64, 1], f32, tag="mean2")
                nc.vector.reduce_sum(mean[:], y2[:], axis=mybir.AxisListType.X)
                nc.vector.tensor_scalar_mul(mean[:], mean[:], 1.0 / 1024.0)
                sq = fin.tile([64, 1024], f32, tag="sq2")
                sqs = fin.tile([64, 1], f32, tag="sqs2")
                nc.scalar.activation(sq[:], y2[:], AF.Square, accum_out=sqs[:])
                m2 = fin.tile([64, 1], f32, tag="m22")
                nc.scalar.activation(m2[:], mean[:], AF.Square)
                var = fin.tile([64, 1], f32, tag="var2")
                nc.vector.tensor_scalar(var[:], sqs[:], 1.0 / 1024.0, EPS,
                                        op0=ALU.mult, op1=ALU.add)
                nc.vector.tensor_tensor(var[:], var[:], m2[:],
                                        op=ALU.subtract)
                std = fin.tile([64, 1], f32, tag="std2")
                nc.scalar.activation(std[:], var[:], AF.Sqrt)
                rstd = fin.tile([64, 1], f32, tag="rstd2")
                nc.vector.reciprocal(rstd[:], std[:])
                on = fin.tile([64, 1024], f32, tag="on", name=f"on{g}")
                nc.vector.tensor_scalar(on[:], y2[:], mean[:], rstd[:],
                                        op0=ALU.subtract, op1=ALU.mult)
                nc.vector.tensor_tensor(on[:], on[:], g2_sb[0:64, :],
                                        op=ALU.mult)
                nc.vector.tensor_tensor(on[:], on[:], bl2_sb[0:64, :],
                                        op=ALU.add)
                nc.sync.dma_start(out_d[g], on[:])
            order = []
            for qq in range(4):
                order += [4 * qq, 4 * qq + 2, 4 * qq + 1, 4 * qq + 3]
            gcount = {g: 0 for g in range(8)}
            DR = mybir.MatmulPerfMode.DoubleRow
            for ci in order:
                hT_c = mck.tile([128, 2048], fp8, tag="hT_c")
                if ci % 2 == 0:
                    for et in range(8):
                        nc.gpsimd.dma_start(
                            hT_c[:, et * 256: et * 256 + 128],
                            hag_outA1[ci // 2, et])
                        nc.gpsimd.dma_start(
                            hT_c[:, et * 256 + 128: et * 256 + 256],
                            hag_outA2[ci // 2, et])
                else:
                    for et in range(8):
                        nc.gpsimd.dma_start(
                            hT_c[:, et * 256: et * 256 + 256],
                            hag_outB[ci // 2, et])
                pcol = mck.tile([128, 2], f32, tag="pcol")
                for th in range(2):
                    lg_ps = ps_eo.tile([128, 8], f32, tag="lg")
                    for p in range(4):
                        nc.tensor.matmul(
                            lg_ps[:],
                            hT_c[:, p * 512: p * 512 + 512]
                            .rearrange("q (k2 t) -> q k2 t", k2=2)
                            [:, :, th * 128: th * 128 + 128],
                            wr_sb[:, p * 16: p * 16 + 16]
                            .rearrange("q (k2 n) -> q k2 n", k2=2),
                            start=(p == 0), stop=False, perf_mode=DR)
                    nc.tensor.matmul(lg_ps[:], ones_row[:, 0:128], br_sb[:],
                                     start=False, stop=True)
                    pe = mck.tile([128, 8], f32, tag="pe")
                    ps = mck.tile([128, 1], f32, tag="ps")
                    nc.scalar.activation(pe[:], lg_ps[:], AF.Exp,
                                         scale=1.0 / 64.0, accum_out=ps[:])
                    ps2 = mck.tile([128, 1], f32, tag="ps2")
                    nc.vector.tensor_scalar_mul(ps2[:], ps[:], 1024.0)
                    pr = mck.tile([128, 1], f32, tag="pr")
                    nc.vector.reciprocal(pr[:], ps2[:])
                    nc.vector.tensor_tensor(pcol[:, th:th + 1], pe[:, 0:1],
                                            pr[:], op=ALU.mult)
                midT = mck.tile([128, 8192], fp8, tag="midT", bufs=2)
                for ft in range(32):
                    md_ps = ps_md.tile([128, 256], f32, tag="md")
                    for p in range(4):
                        nc.tensor.matmul(
                            md_ps[:],
                            w1_sb[:, p * 8192 + ft * 256:
                                  p * 8192 + ft * 256 + 256]
                            .rearrange("q (k2 m) -> q k2 m", k2=2),
                            hT_c[:, p * 512: p * 512 + 512]
                            .rearrange("q (k2 t) -> q k2 t", k2=2),
                            start=(p == 0), stop=(p == 3), perf_mode=DR)
                    nc.scalar.activation(
                        midT[:, ft * 256: ft * 256 + 256], md_ps[:],
                        AF.Relu, bias=b1_sb[:, ft: ft + 1])
                eo_sb = mck.tile([128, 2048], bf, tag="eo", bufs=2)
                for th in range(2):
                    for ec in range(2):
                        eo_ps = ps_eo.tile([128, 512], f32, tag="eo_ps")
                        for qq2 in range(16):
                            nc.tensor.matmul(
                                eo_ps[:],
                                midT[:, qq2 * 512: qq2 * 512 + 512]
                                .rearrange("q (k2 t) -> q k2 t", k2=2)
                                [:, :, th * 128: th * 128 + 128],
                                w2_sb[:, qq2 * 2048: qq2 * 2048 + 2048]
                                .rearrange("q (k2 e) -> q k2 e", k2=2)
                                [:, :, ec * 512: ec * 512 + 512],
                                start=(qq2 == 0), stop=False, perf_mode=DR)
                        nc.tensor.matmul(
                            eo_ps[:], ones_row[:, 0:128],
                            b2_sb[:, ec * 512: ec * 512 + 512],
                            start=False, stop=True)
                        if ec == 0:
                            nc.scalar.activation(
                                eo_sb[:, th * 1024 + ec * 512:
                                      th * 1024 + ec * 512 + 512],
                                eo_ps[:], AF.Identity,
                                scale=pcol[:, th: th + 1])
                        else:
                            nc.vector.tensor_scalar_mul(
                                eo_sb[:, th * 1024 + ec * 512:
                                      th * 1024 + ec * 512 + 512],
                                eo_ps[:], pcol[:, th: th + 1])
                g, gi = ci // 2, ci % 2
                for th in range(2):
                    nc.sync.dma_start(
                        rs_ins[g][gi * 256 + th * 128: gi * 256 + th * 128 + 128, :],
                        eo_sb[:, th * 1024: th * 1024 + 1024])
                gcount[g] += 1
                if gcount[g] == 2:
                    nc.gpsimd.collective_compute(
                        "ReduceScatter", mybir.AluOpType.add,
                        replica_groups=[list(range(8))],
                        ins=[rs_ins[g].opt()], outs=[rs_outs[g].opt()])
                    if g >= 2:
                        do_fin(g - 2)
            do_fin(6)
            do_fin(7)
        moe_cm.__exit__(None, None, None)

        cpool_cm.__exit__(None, None, None)
    nc.compile()
    return nc


def _prep_inputs(inputs):
    f = np.float32
    x = np.asarray(inputs["x"], f)
    wq, bq = np.asarray(inputs["wq"], f), np.asarray(inputs["bq"], f)
    wk, bk = np.asarray(inputs["wk"], f), np.asarray(inputs["bk"], f)
    wv, bv = np.asarray(inputs["wv"], f), np.asarray(inputs["bv"], f)
    wp, bp = np.asarray(inputs["wp"], f), np.asarray(inputs["bp"], f)
    ln1_g, ln1_b = np.asarray(inputs["ln1_g"], f), np.asarray(inputs["ln1_b"], f)
    ln2_g, ln2_b = np.asarray(inputs["ln2_g"], f), np.asarray(inputs["ln2_b"], f)
    wr, br = np.asarray(inputs["wr"], f), np.asarray(inputs["br"], f)
    w1, b1 = np.asarray(inputs["w1"], f), np.asarray(inputs["b1"], f)
    w2, b2 = np.asarray(inputs["w2"], f), np.asarray(inputs["b2"], f)

    def etile(a):  # [E, M] -> [128, 8*M]
        M = a.shape[1]
        return np.ascontiguousarray(
            a.reshape(8, 128, M).transpose(1, 0, 2).reshape(128, 8 * M))

    wq_f = wq.transpose(1, 0, 2).reshape(E, E)   # [e, h*64+d]
    wk_f = wk.transpose(1, 0, 2).reshape(E, E)
    wv_f = wv.transpose(1, 0, 2).reshape(E, E)
    # sect-major [k|v|q], each section e-tiled: contiguous section DMAs
    wqkv_t = np.concatenate(
        [etile(64.0 * wk_f), etile(64.0 * wv_f), etile(64.0 * wq_f)],
        axis=1).astype(F8)                                   # [128, 3*8192]
    bqk = 64.0 * np.concatenate(
        [bq.reshape(-1).reshape(8, 128).T,
         bk.reshape(-1).reshape(8, 128).T], axis=1).astype(f)
    wp_t = etile(64.0 * wp).astype(F8)                       # [128, 8*1024]
    w1e = (ln1_g[:, None] * w1).astype(f)                    # [n,E,FF]
    b1e = b1 + ln1_b @ w1                                    # [n,FF]
    wre = (ln1_g[:, None] * wr).astype(f)                    # [E,8]
    bre = br + ln1_b @ wr                                    # [8]
    ident = np.eye(128, dtype=BF16)

    in_maps = []
    for r in range(NCORE):
        b, p = r // 2, r % 2
        # interleaved token assignment: local s_loc <-> orig row 2*s_loc + p
        xs = np.ascontiguousarray(x[b, p::2, :])             # [512, E]
        xpart = np.ascontiguousarray(x[b, 1 - p::2, :])      # partner tokens
        xT_t = np.concatenate(
            [etile(np.ascontiguousarray(xs.T)),
             etile(np.ascontiguousarray(xpart.T))], axis=1).astype(F8)
        xr_t = np.ascontiguousarray(
            xs.reshape(4, 128, 1024).transpose(1, 0, 2).reshape(128, 4096), f)
        # final-phase x rows: per src-core group g, local tokens [64r,64r+64)
        sv2 = 64 * r + np.arange(64)
        xr2_t = np.ascontiguousarray(np.concatenate(
            [x[g // 2, 2 * sv2 + (g % 2), :] for g in range(8)], axis=1), f)
        # diagonal causal masks: half0 = own parity keys, half1 = partner
        masks = np.zeros((4, 128, 256), BF16)
        ti = np.arange(128)
        sj = np.arange(128)
        own = (ti[:, None] <= sj[None, :])
        part = (ti[:, None] <= sj[None, :]) if p == 1 else                (ti[:, None] < sj[None, :])
        for ss in range(4):
            masks[ss][:, 0:128] = own.astype(BF16)
            masks[ss][:, 128:256] = part.astype(BF16)
        perm = [r] + [i for i in range(NEXP) if i != r]
        # fp8 DoubleRow layouts; w1 x16, w2/wr x64 to clear e4m3 subnormals
        wr_p = etile(64.0 * wre[:, perm]).astype(F8)         # [128, 8*8]
        br_p = (64.0 * bre[perm]).reshape(1, 8).astype(BF16)
        w1_t = np.ascontiguousarray(
            (16.0 * w1e[r]).reshape(4, 2, 128, 32, 128)
            .transpose(2, 0, 3, 1, 4).reshape(128, 32768)).astype(F8)
        b1_t = np.ascontiguousarray(
            16.0 * b1e[r].reshape(32, 128).T, f)             # [128, 32]
        w2_t = np.ascontiguousarray(
            (64.0 * w2[r]).reshape(16, 2, 128, 1024)
            .transpose(2, 0, 1, 3).reshape(128, 32768)).astype(F8)
        in_maps.append({
            "xT": xT_t, "xr": xr_t, "xr2": xr2_t, "wqkv": wqkv_t, "bqk": bqk,
            "bv": (64.0 * bv).reshape(1, E).astype(BF16),
            "wp": wp_t, "bp": (64.0 * bp).reshape(1, E).astype(BF16),
            "masks": masks, "ident": ident,
            "w1": w1_t, "b1": b1_t, "w2": w2_t,
            "b2": (1024.0 * b2[r]).reshape(1, E).astype(BF16),
            "wr": wr_p, "br": br_p,
            "g2": np.broadcast_to(ln2_g, (128, E)).astype(f).copy(),
            "bl2": np.broadcast_to(ln2_b, (128, E)).astype(f).copy(),
        })
    return in_maps


def kernel(**inputs):
    from concourse import bass_utils
    if "nc" not in _CACHE:
        _CACHE["nc"] = _build_program()
    nc = _CACHE["nc"]
    in_maps = _prep_inputs(inputs)
    res = bass_utils.run_bass_kernel_spmd(
        nc, in_maps, core_ids=list(range(NCORE)))
    # core r returns, per src-core group g, that core's local tokens
    # [64r, 64r+64) -> batch g//2, orig rows 2*s + (g%2)
    full = np.empty((B, S, E), np.float32)
    for r in range(NCORE):
        o = res.results[r]["out"]                            # [8, 64, 1024]
        sv2 = 64 * r + np.arange(64)
        for g in range(NCORE):
            full[g // 2, 2 * sv2 + (g % 2), :] = o[g]
    return full



# revision 56
# speedup vs baseline: 1.0438x; 1.0438x over previous
"""Trainium2 Bass kernel for nn_Block_56616258896419 (moe_routing).

Self-contained: takes FULL inputs (as from setup_inputs()), returns FULL
[4,1024,1024] f32 output. Internally shards across 8 NeuronCores:
  - tokens 8-way (core r: batch r//2, sequence half r%2) for attention/LN
  - experts 8-way (core r computes expert r over ALL tokens) for the MoE
Collectives: pairwise AllGather of K/V, 8-way AllGather of LN1'd
activations (transposed, bf16), 4x chunked 8-way ReduceScatter of the
prob-weighted expert outputs (overlapped with MoE compute).
"""
import numpy as np
import ml_dtypes

B, S, E, H, HD, NEXP, FF = 4, 1024, 1024, 16, 64, 8, 4096
NCORE = 8
TOK = 512          # tokens per core
TC = 256           # MoE token-chunk
NCHUNK = (B * S) // TC
EPS = 1e-5
BF16 = ml_dtypes.bfloat16
F8 = ml_dtypes.float8_e4m3

_CACHE = {}


def _build_program():
    import concourse.bacc as bacc
    import concourse.mybir as mybir
    import concourse.tile as tile

    dt = mybir.dt
    f32, bf = dt.float32, dt.bfloat16
    AF = mybir.ActivationFunctionType
    ALU = mybir.AluOpType

    nc = bacc.Bacc("TRN2", target_bir_lowering=False, debug=False,
                   num_devices=NCORE)

    # ---------------- I/O ----------------
    def inp(name, shape, d):
        return nc.dram_tensor(name, shape, d, kind="ExternalInput").ap()

    fp8i = dt.float8e4
    xT_d = inp("xT", [128, 2 * 4096], fp8i)       # x^T [own|partner], e-tiled
    xr_d = inp("xr", [128, 4 * 1024], f32)        # x token-major, tt blocks
    xr2_d = inp("xr2", [64, 8 * 1024], f32)       # x rows per src-core group
    wqkv_d = inp("wqkv", [128, 8 * 3072], fp8i)   # [E,3E] e-tiled, x64
    bqk_d = inp("bqk", [128, 16], f32)            # x64
    bv_d = inp("bv", [1, 1024], bf)               # x64
    wp_d = inp("wp", [128, 8 * 1024], fp8i)       # x64
    bp_d = inp("bp", [1, 1024], bf)               # x64
    masks_d = inp("masks", [4, 128, 256], bf)     # diag-pair 0/1 masks
    ident_d = inp("ident", [128, 128], bf)
    fp8 = dt.float8e4
    w1_d = inp("w1", [128, 8 * 4096], fp8)        # ln1-folded, x16, DR pairs
    b1_d = inp("b1", [128, 32], f32)              # x16
    w2_d = inp("w2", [128, 32 * 1024], fp8)       # x64, DR pairs
    b2_d = inp("b2", [1, 1024], bf)               # x1024
    wr_d = inp("wr", [128, 8 * 8], fp8)           # ln1-folded, permuted, x64
    br_d = inp("br", [1, 8], bf)
    g2_d = inp("g2", [128, 1024], f32)            # ln2_g replicated
    bl2_d = inp("bl2", [128, 1024], f32)          # ln2_b replicated
    out_d = nc.dram_tensor("out", [8, 64, 1024], f32,
                           kind="ExternalOutput").ap()

    # ---------------- internal DRAM ----------------
    warm_in = nc.dram_tensor("warm_in", [1, 64], fp8).ap()
    warm_out = nc.dram_tensor("warm_out", [8, 1, 64], fp8,
                              addr_space="Shared").ap()
    hag_inA1 = nc.dram_tensor("hag_inA1", [8, 128, 128], fp8).ap()
    hag_inA2 = nc.dram_tensor("hag_inA2", [8, 128, 128], fp8).ap()
    hag_inB = nc.dram_tensor("hag_inB", [8, 128, 256], fp8).ap()
    hag_outA1 = nc.dram_tensor("hag_outA1", [8, 8, 128, 128], fp8,
                               addr_space="Shared").ap()
    hag_outA2 = nc.dram_tensor("hag_outA2", [8, 8, 128, 128], fp8,
                               addr_space="Shared").ap()
    hag_outB = nc.dram_tensor("hag_outB", [8, 8, 128, 256], fp8,
                              addr_space="Shared").ap()
    rs_ins = [nc.dram_tensor(f"rs_in{g}", [512, 1024], bf).ap()
              for g in range(8)]
    rs_outs = [nc.dram_tensor(f"rs_out{g}", [64, 1024], bf).ap()
               for g in range(8)]

    with tile.TileContext(nc) as tc:
        cpool_cm = tc.tile_pool(name="cpool", bufs=1, side="left")
        cpool = cpool_cm.__enter__()
        ones_row = cpool.tile([1, 128], bf)
        nc.vector.memset(ones_row[:], 1.0)
        ones_f = cpool.tile([1, 128], f32)
        nc.vector.memset(ones_f[:], 1.0)
        bqk_sb = cpool.tile([128, 16], f32)
        nc.sync.dma_start(bqk_sb[:], bqk_d[:])
        bv_sb = cpool.tile([1, 1024], bf)
        nc.sync.dma_start(bv_sb[:], bv_d[:])
        bp_sb = cpool.tile([1, 1024], bf)
        nc.sync.dma_start(bp_sb[:], bp_d[:])
        ident_sb = cpool.tile([128, 128], bf)
        nc.sync.dma_start(ident_sb[:], ident_d[:])
        wr_sb = cpool.tile([128, 64], fp8)
        nc.sync.dma_start(wr_sb[:], wr_d[:])
        br_sb = cpool.tile([1, 8], bf)
        nc.sync.dma_start(br_sb[:], br_d[:])
        b1_sb = cpool.tile([128, 32], f32)
        nc.sync.dma_start(b1_sb[:], b1_d[:])
        b2_sb = cpool.tile([1, 1024], bf)
        nc.sync.dma_start(b2_sb[:], b2_d[:])

        # ===== phase 1: local K/V for BOTH interleave-halves, then Q =====
        # all fp8 DoubleRow over e-tile pairs; weights x64, descaled on exit
        DR = mybir.MatmulPerfMode.DoubleRow
        qkv_cm = tc.tile_pool(name="qkv", bufs=1, side="right")
        qkv = qkv_cm.__enter__()
        xT_sb = qkv.tile([128, 8192], fp8)
        nc.sync.dma_start(xT_sb[:, 0:4096], xT_d[:, 0:4096])
        wqkv_sb = qkv.tile([128, 24576], fp8)
        # sect-major host layout [k|v|q]; spread across rings so the first
        # K matmul waits only ~1.5MB: K on scalar ring, V on gpsimd ring
        nc.scalar.dma_start(wqkv_sb[:, 0:8192], wqkv_d[:, 0:8192])
        nc.gpsimd.dma_start(wqkv_sb[:, 8192:16384], wqkv_d[:, 8192:16384])
        nc.sync.dma_start(xT_sb[:, 4096:8192], xT_d[:, 4096:8192])
        nc.sync.dma_start(wqkv_sb[:, 16384:24576], wqkv_d[:, 16384:24576])
        xT_v = xT_sb[:].rearrange("p (h e t) -> p h e t", h=2, e=8)
        wq_v = wqkv_sb[:].rearrange("p (s e c) -> p s e c", s=3, e=8)

        attn_cm = tc.tile_pool(name="attn", bufs=1, side="left")
        attn = attn_cm.__enter__()
        qT_sb = attn.tile([128, 4096], bf)
        kT_full = attn.tile([128, 8192], bf)     # [j][half*512 + s]
        v_full = attn.tile([128, 8192], bf)      # [u = half*4+tt][hd]
        # augmented V: per t-tile, 16 heads x (64 v-cols + 1 ones-col);
        # ones-cols set up front, v copied per u-tile inside the V loop
        v_aug = attn.tile([128, 8 * 1040], bf)
        for tt in range(8):
            nc.vector.memset(
                v_aug[:, tt * 1040: tt * 1040 + 1040]
                .rearrange("p (h dd) -> p h dd", dd=65)[:, :, 64:65], 1.0)

        with tc.tile_pool(name="ps_qkv", bufs=3, space="PSUM") as psq:
            for half in range(2):
                for j in range(8):
                    k_ps = psq.tile([128, 512], f32, tag="qk_ps")
                    for p in range(4):
                        nc.tensor.matmul(
                            k_ps[:],
                            wq_v[:, 0, 2 * p: 2 * p + 2,
                                 j * 128: j * 128 + 128],
                            xT_v[:, half, 2 * p: 2 * p + 2, :],
                            start=(p == 0), stop=(p == 3), perf_mode=DR)
                    nc.vector.tensor_scalar(
                        kT_full[:, j * 1024 + half * 512:
                                j * 1024 + half * 512 + 512], k_ps[:],
                        bqk_sb[:, 8 + j: 8 + j + 1], 1.0 / 64.0,
                        op0=ALU.add, op1=ALU.mult)
            # qT next (own tokens = half 0) so scores can start right away
            for j in range(8):
                q_ps = psq.tile([128, 512], f32, tag="qk_ps")
                for p in range(4):
                    nc.tensor.matmul(
                        q_ps[:],
                        wq_v[:, 2, 2 * p: 2 * p + 2, j * 128: j * 128 + 128],
                        xT_v[:, 0, 2 * p: 2 * p + 2, :],
                        start=(p == 0), stop=(p == 3), perf_mode=DR)
                nc.vector.tensor_scalar(
                    qT_sb[:, j * 512: j * 512 + 512], q_ps[:],
                    bqk_sb[:, j: j + 1], 0.125 / 64.0,
                    op0=ALU.add, op1=ALU.mult)
            for half in range(2):
                for tt in range(4):
                    u = half * 4 + tt
                    for c in range(2):
                        v_ps = psq.tile([128, 512], f32, tag="v_ps")
                        for p in range(4):
                            nc.tensor.matmul(
                                v_ps[:],
                                xT_v[:, half, 2 * p: 2 * p + 2,
                                     tt * 128: tt * 128 + 128],
                                wq_v[:, 1, 2 * p: 2 * p + 2,
                                     c * 512: c * 512 + 512],
                                start=(p == 0), stop=False, perf_mode=DR)
                        nc.tensor.matmul(
                            v_ps[:], ones_row[:, 0:128],
                            bv_sb[:, c * 512: c * 512 + 512],
                            start=False, stop=True)
                        nc.scalar.activation(
                            v_full[:, u * 1024 + c * 512:
                                   u * 1024 + c * 512 + 512], v_ps[:],
                            AF.Identity, scale=1.0 / 64.0)
                    nc.vector.tensor_copy(
                        v_aug[:, u * 1040: u * 1040 + 1040]
                        .rearrange("p (h dd) -> p h dd", dd=65)[:, :, 0:64],
                        v_full[:, u * 1024: u * 1024 + 1024]
                        .rearrange("p (h dd) -> p h dd", dd=64))
        qkv_cm.__exit__(None, None, None)

        # MoE weights prefetch on the vector ring (overlaps attention);
        # wp/x residual loads early on the sync ring so proj never waits
        moe_cm = tc.tile_pool(name="moe", bufs=1, side="right")
        moe = moe_cm.__enter__()
        w1_sb = moe.tile([128, 32768], fp8)
        w2_sb = moe.tile([128, 32768], fp8)
        for et in range(8):
            nc.scalar.dma_start(w1_sb[:, et * 4096: et * 4096 + 4096],
                                w1_d[:, et * 4096: et * 4096 + 4096])
        for ft8 in range(8):
            nc.scalar.dma_start(w2_sb[:, ft8 * 4096: ft8 * 4096 + 4096],
                                w2_d[:, ft8 * 4096: ft8 * 4096 + 4096])

        # warm the collective channel so the first real AllGather doesn't
        # pay rendezvous/ring-warmup latency on the critical path
        nc.gpsimd.collective_compute(
            "AllGather", mybir.AluOpType.bypass,
            replica_groups=[list(range(8))],
            ins=[warm_in.opt()], outs=[warm_out.opt()])

        # ============ phase 2: attention ============
        mask_sb = attn.tile([128, 1024], bf)
        for ss in range(4):
            nc.sync.dma_start(mask_sb[:, ss * 256: ss * 256 + 256],
                              masks_d[ss])

        proj_cm = tc.tile_pool(name="proj", bufs=1, side="right")
        projp = proj_cm.__enter__()
        catT_sb = projp.tile([128, 4096], bf)    # unnormalized heads^T
        catT8 = projp.tile([128, 4096], fp8)     # normalized, fp8 for proj
        wp_sb = projp.tile([128, 8192], fp8)
        nc.sync.dma_start(wp_sb[:], wp_d[:])
        x_sb = projp.tile([128, 4096], f32)
        nc.sync.dma_start(x_sb[:], xr_d[:])

        with tc.tile_pool(name="sc", bufs=4, side="left") as scp, \
             tc.tile_pool(name="ps_sc", bufs=2, space="PSUM") as ps_sc, \
             tc.tile_pool(name="ps_av", bufs=4, space="PSUM") as ps_av:
            # one N-restricted matmul per key-tile: keys (ph,tl) attend to
            # queries >= tl, so scores/exp/AV each run [128, (4-tl)*128].
            # PE emission is software-pipelined: scores run one key-tile
            # ahead of AV (covering exp latency), and each j's epilogue
            # (softmax-normalize via K=1 matmuls) is deferred into j+1.

            def epilogue(j, avps):
                # partition-shifting copies (0:64 -> 64:128, row 64 -> 0)
                # must run on ACT; lane-aligned ones go to DVE
                rcs = []
                for h01 in range(2):
                    if h01 == 0:
                        nc.vector.tensor_copy(
                            catT_sb[0:64, j * 512: j * 512 + 512],
                            avps[0][0:64, :])
                    else:
                        nc.scalar.copy(
                            catT_sb[64:128, j * 512: j * 512 + 512],
                            avps[1][0:64, :])
                    sm_f = scp.tile([1, 512], f32, tag="sm_f", bufs=4,
                                    name=f"smf{j}_{h01}")
                    nc.scalar.copy(sm_f[:], avps[h01][64:65, :])
                    rc_f = scp.tile([1, 512], f32, tag="rc_f", bufs=4,
                                    name=f"rcf{j}_{h01}")
                    nc.vector.reciprocal_approx_fast(rc_f[:], sm_f[:])
                    rc_h = scp.tile([1, 512], bf, tag="rc_h", bufs=4,
                                    name=f"rc{j}_{h01}")
                    nc.vector.tensor_copy(rc_h[:], rc_f[:])
                    rcs.append(rc_h)
                bc_ps = ps_sc.tile([128, 1024], f32, tag="sc",
                                   name=f"bc{j}")[:, 0:512]
                nc.tensor.matmul(bc_ps[0:64, :], ones_row[:, 0:64],
                                 rcs[0][:], start=True, stop=True)
                nc.tensor.matmul(bc_ps[64:128, :], ones_row[:, 0:64],
                                 rcs[1][:], start=True, stop=True)
                nc.vector.tensor_tensor(
                    catT_sb[:, j * 512: j * 512 + 512],
                    catT_sb[:, j * 512: j * 512 + 512], bc_ps[:],
                    op=ALU.mult)
                nc.vector.tensor_copy(catT8[:, j * 512: j * 512 + 512],
                                      catT_sb[:, j * 512: j * 512 + 512])

            # scores packed in (tl, tl+1) pairs -> one exp per pair; AV
            # runs one pair behind so exp latency is always covered
            pend = None
            for j in range(8):
                avps = [ps_av.tile([65, 512], f32, tag="av", name=f"av{j}_{k}")
                        for k in range(2)]
                for h01 in range(2):
                    po = 64 * h01
                    prev = None
                    first = True

                    def emit_av(pavs, start):
                        for pqo, pu, pexp, plo, pN in pavs:
                            nc.tensor.matmul(
                                avps[h01][:, pqo:512],
                                v_aug[:, pu * 1040 + (2 * j + h01) * 65:
                                      pu * 1040 + (2 * j + h01) * 65 + 65],
                                pexp[:, plo:plo + pN],
                                start=start, stop=False)
                            start = False

                    for ph in range(2):
                        for ta in (0, 2):
                            Na = (4 - ta) * 128
                            Nb = (4 - ta - 1) * 128
                            sc_ps = ps_sc.tile([128, 1024], f32, tag="sc",
                                               name=f"sc{j}_{h01}_{ph}_{ta}")
                            nc.tensor.matmul(
                                sc_ps[:, 0:Na],
                                kT_full[po:po + 64,
                                        j * 1024 + ph * 512 + ta * 128:
                                        j * 1024 + ph * 512 + ta * 128 + 128],
                                qT_sb[po:po + 64,
                                      j * 512 + ta * 128: j * 512 + 512],
                                start=True, stop=True)
                            nc.tensor.matmul(
                                sc_ps[:, Na:Na + Nb],
                                kT_full[po:po + 64,
                                        j * 1024 + ph * 512 + ta * 128 + 128:
                                        j * 1024 + ph * 512 + ta * 128 + 256],
                                qT_sb[po:po + 64,
                                      j * 512 + ta * 128 + 128: j * 512 + 512],
                                start=True, stop=True)
                            expT = scp.tile([128, 1024], bf, tag="expT",
                                            name=f"ex{j}_{h01}_{ph}_{ta}")
                            nc.scalar.activation(
                                expT[:, 0:Na + Nb], sc_ps[:, 0:Na + Nb],
                                AF.Exp)
                            nc.vector.tensor_tensor(
                                expT[:, 0:128], expT[:, 0:128],
                                mask_sb[:, ta * 256 + ph * 128:
                                        ta * 256 + ph * 128 + 128],
                                op=ALU.mult)
                            nc.vector.tensor_tensor(
                                expT[:, Na:Na + 128], expT[:, Na:Na + 128],
                                mask_sb[:, (ta + 1) * 256 + ph * 128:
                                        (ta + 1) * 256 + ph * 128 + 128],
                                op=ALU.mult)
                            if prev is not None:
                                emit_av(prev, first)
                                first = False
                            if pend is not None and h01 == 0 and \
                                    ph == 1 and ta == 0:
                                epilogue(*pend)
                                pend = None
                            u = ph * 4 + ta
                            prev = [(ta * 128, u, expT, 0, Na),
                                    ((ta + 1) * 128, u + 1, expT, Na, Nb)]
                    emit_av(prev[:1], first)
                    pqo, pu, pexp, plo, pN = prev[1]
                    nc.tensor.matmul(
                        avps[h01][:, pqo:512],
                        v_aug[:, pu * 1040 + (2 * j + h01) * 65:
                              pu * 1040 + (2 * j + h01) * 65 + 65],
                        pexp[:, plo:plo + pN],
                        start=False, stop=True)
                pend = (j, avps)
            epilogue(*pend)
        attn_cm.__exit__(None, None, None)

        h_sb = projp.tile([128, 4096], bf)
        hT_stage = projp.tile([128, 4096], fp8)
        cat_v = catT8[:].rearrange("p (j t) -> p j t", j=8)
        wp_v = wp_sb[:].rearrange("p (j e) -> p j e", j=8)

        with tc.tile_pool(name="prw", bufs=2, side="left") as prp, \
             tc.tile_pool(name="ps_pr", bufs=4, space="PSUM") as ps_pr:
            for tt in range(4):
                y_sb = prp.tile([128, 1024], f32, tag="y")
                for ec in range(2):
                    ao_ps = ps_pr.tile([128, 512], f32, tag="ao")
                    for p in range(4):
                        nc.tensor.matmul(
                            ao_ps[:],
                            cat_v[:, 2 * p: 2 * p + 2,
                                  tt * 128: tt * 128 + 128],
                            wp_v[:, 2 * p: 2 * p + 2,
                                 ec * 512: ec * 512 + 512],
                            start=(p == 0), stop=False, perf_mode=DR)
                    nc.tensor.matmul(
                        ao_ps[:], ones_row[:, 0:128],
                        bp_sb[:, ec * 512: ec * 512 + 512],
                        start=False, stop=True)
                    nc.vector.tensor_scalar(
                        y_sb[:, ec * 512: ec * 512 + 512], ao_ps[:],
                        1.0 / 64.0, None, op0=ALU.mult)
                    nc.vector.tensor_tensor(
                        y_sb[:, ec * 512: ec * 512 + 512],
                        y_sb[:, ec * 512: ec * 512 + 512],
                        x_sb[:, tt * 1024 + ec * 512: tt * 1024 + ec * 512 + 512],
                        op=ALU.add)
                # LN1 stats
                mean = prp.tile([128, 1], f32, tag="mean")
                nc.vector.reduce_sum(mean[:], y_sb[:], axis=mybir.AxisListType.X)
                nc.vector.tensor_scalar_mul(mean[:], mean[:], 1.0 / 1024.0)
                sq = prp.tile([128, 1024], bf, tag="sq")
                sqs = prp.tile([128, 1], f32, tag="sqs")
                nc.scalar.activation(sq[:], y_sb[:], AF.Square,
                                     accum_out=sqs[:])
                m2 = prp.tile([128, 1], f32, tag="m2")
                nc.scalar.activation(m2[:], mean[:], AF.Square)
                var = prp.tile([128, 1], f32, tag="var")
                nc.vector.tensor_scalar(var[:], sqs[:], 1.0 / 1024.0, EPS,
                                        op0=ALU.mult, op1=ALU.add)
                nc.vector.tensor_tensor(var[:], var[:], m2[:], op=ALU.subtract)
                std = prp.tile([128, 1], f32, tag="std")
                nc.scalar.activation(std[:], var[:], AF.Sqrt)
                rstd = prp.tile([128, 1], f32, tag="rstd")
                nc.vector.reciprocal(rstd[:], std[:])
                nc.vector.tensor_scalar(
                    h_sb[:, tt * 1024: tt * 1024 + 1024], y_sb[:],
                    mean[:], rstd[:], op0=ALU.subtract, op1=ALU.mult)
                # transpose h tile -> hT
                for et in range(8):
                    tp = ps_pr.tile([128, 128], bf, tag="tp")
                    nc.tensor.transpose(
                        tp[:], h_sb[:, tt * 1024 + et * 128:
                                    tt * 1024 + et * 128 + 128], ident_sb[:])
                    nc.scalar.copy(
                        hT_stage[:, et * 512 + tt * 128:
                                 et * 512 + tt * 128 + 128], tp[:])
                if tt == 0:
                    for et in range(8):
                        nc.gpsimd.dma_start(
                            hag_inA1[et],
                            hT_stage[:, et * 512: et * 512 + 128])
                    nc.gpsimd.collective_compute(
                        "AllGather", mybir.AluOpType.bypass,
                        replica_groups=[list(range(8))],
                        ins=[hag_inA1.opt()], outs=[hag_outA1.opt()])
                if tt == 1:
                    for et in range(8):
                        nc.gpsimd.dma_start(
                            hag_inA2[et],
                            hT_stage[:, et * 512 + 128: et * 512 + 256])
                    nc.gpsimd.collective_compute(
                        "AllGather", mybir.AluOpType.bypass,
                        replica_groups=[list(range(8))],
                        ins=[hag_inA2.opt()], outs=[hag_outA2.opt()])
                if tt == 3:
                    for et in range(8):
                        nc.gpsimd.dma_start(
                            hag_inB[et],
                            hT_stage[:, et * 512 + 256: et * 512 + 512])
                    nc.gpsimd.collective_compute(
                        "AllGather", mybir.AluOpType.bypass,
                        replica_groups=[list(range(8))],
                        ins=[hag_inB.opt()], outs=[hag_outB.opt()])
            # (b) PE warm-keepers over the h-AG wait
            for wi in range(16):
                wm_ps = ps_pr.tile([128, 512], f32, tag="ao", name=f"wm{wi}")
                nc.tensor.matmul(wm_ps[:], catT8[:, 0:128],
                                 wp_sb[:, 0:512], start=True, stop=True)
        proj_cm.__exit__(None, None, None)

        # ============ phase 3: MoE (expert r over all tokens) ============
        # fin (residual+LN2) work is interleaved per completed RS group so
        # only the last group's LN2 sits on the tail
        with tc.tile_pool(name="fin", bufs=2, side="left") as fin, \
             tc.tile_pool(name="mchunk", bufs=2, side="left") as mck, \
             tc.tile_pool(name="ps_md", bufs=2, space="PSUM") as ps_md, \
             tc.tile_pool(name="ps_eo", bufs=3, space="PSUM") as ps_eo:
            x2_sb = fin.tile([64, 8192], f32, bufs=1)
            nc.sync.dma_start(x2_sb[:], xr2_d[:])
            g2_sb = fin.tile([128, 1024], f32, bufs=1)
            nc.sync.dma_start(g2_sb[:], g2_d[:])
            bl2_sb = fin.tile([128, 1024], f32, bufs=1)
            nc.sync.dma_start(bl2_sb[:], bl2_d[:])

            def do_fin(g):
                y2b = fin.tile([64, 1024], bf, tag="y2b", name=f"y2b{g}")
                nc.sync.dma_start(y2b[:], rs_outs[g][:])
                y2 = fin.tile([64, 1024], f32, tag="y2", name=f"y2{g}")
                nc.vector.tensor_tensor(
                    y2[:], y2b[:], x2_sb[:, g * 1024: g * 1024 + 1024],
                    op=ALU.add)
                mean = fin.tile([64, 1], f32, tag="mean2")
                nc.vector.reduce_sum(mean[:], y2[:], axis=mybir.AxisListType.X)
                nc.vector.tensor_scalar_mul(mean[:], mean[:], 1.0 / 1024.0)
                sq = fin.tile([64, 1024], f32, tag="sq2")
                sqs = fin.tile([64, 1], f32, tag="sqs2")
                nc.scalar.activation(sq[:], y2[:], AF.Square, accum_out=sqs[:])
                m2 = fin.tile([64, 1], f32, tag="m22")
                nc.scalar.activation(m2[:], mean[:], AF.Square)
                var = fin.tile([64, 1], f32, tag="var2")
                nc.vector.tensor_scalar(var[:], sqs[:], 1.0 / 1024.0, EPS,
                                        op0=ALU.mult, op1=ALU.add)
                nc.vector.tensor_tensor(var[:], var[:], m2[:],
                                        op=ALU.subtract)
                std = fin.tile([64, 1], f32, tag="std2")
                nc.scalar.activation(std[:], var[:], AF.Sqrt)
                rstd = fin.tile([64, 1], f32, tag="rstd2")
                nc.vector.reciprocal(rstd[:], std[:])
                on = fin.tile([64, 1024], f32, tag="on", name=f"on{g}")
                nc.vector.tensor_scalar(on[:], y2[:], mean[:], rstd[:],
                                        op0=ALU.subtract, op1=ALU.mult)
                nc.vector.tensor_tensor(on[:], on[:], g2_sb[0:64, :],
                                        op=ALU.mult)
                nc.vector.tensor_tensor(on[:], on[:], bl2_sb[0:64, :],
                                        op=ALU.add)
                nc.sync.dma_start(out_d[g], on[:])
            DR = mybir.MatmulPerfMode.DoubleRow
            # 512-token chunks: ck<4 pairs two cores' A-halves (available
            # right after the early AllGathers), ck>=4 pairs their B-halves
            for ck in range(8):
                hT_c = mck.tile([128, 4096], fp8, tag="hT_c")
                if ck < 4:
                    s0, s1 = 2 * ck, 2 * ck + 1
                    for et in range(8):
                        nc.gpsimd.dma_start(
                            hT_c[:, et * 512: et * 512 + 128],
                            hag_outA1[s0, et])
                        nc.gpsimd.dma_start(
                            hT_c[:, et * 512 + 128: et * 512 + 256],
                            hag_outA2[s0, et])
                        nc.gpsimd.dma_start(
                            hT_c[:, et * 512 + 256: et * 512 + 384],
                            hag_outA1[s1, et])
                        nc.gpsimd.dma_start(
                            hT_c[:, et * 512 + 384: et * 512 + 512],
                            hag_outA2[s1, et])
                else:
                    s0, s1 = 2 * (ck - 4), 2 * (ck - 4) + 1
                    for et in range(8):
                        nc.gpsimd.dma_start(
                            hT_c[:, et * 512: et * 512 + 256],
                            hag_outB[s0, et])
                        nc.gpsimd.dma_start(
                            hT_c[:, et * 512 + 256: et * 512 + 512],
                            hag_outB[s1, et])
                pcol = mck.tile([128, 4], f32, tag="pcol")
                for th in range(4):
                    lg_ps = ps_eo.tile([128, 8], f32, tag="lg")
                    for p in range(4):
                        nc.tensor.matmul(
                            lg_ps[:],
                            hT_c[:, p * 1024: p * 1024 + 1024]
                            .rearrange("q (k2 t) -> q k2 t", k2=2)
                            [:, :, th * 128: th * 128 + 128],
                            wr_sb[:, p * 16: p * 16 + 16]
                            .rearrange("q (k2 n) -> q k2 n", k2=2),
                            start=(p == 0), stop=False, perf_mode=DR)
                    nc.tensor.matmul(lg_ps[:], ones_row[:, 0:128], br_sb[:],
                                     start=False, stop=True)
                    pe = mck.tile([128, 8], f32, tag="pe")
                    ps = mck.tile([128, 1], f32, tag="ps")
                    nc.scalar.activation(pe[:], lg_ps[:], AF.Exp,
                                         scale=1.0 / 64.0, accum_out=ps[:])
                    ps2 = mck.tile([128, 1], f32, tag="ps2")
                    nc.vector.tensor_scalar_mul(ps2[:], ps[:], 1024.0)
                    pr = mck.tile([128, 1], f32, tag="pr")
                    nc.vector.reciprocal(pr[:], ps2[:])
                    nc.vector.tensor_tensor(pcol[:, th:th + 1], pe[:, 0:1],
                                            pr[:], op=ALU.mult)
                midT = mck.tile([128, 16384], fp8, tag="midT", bufs=2)
                for ft in range(32):
                    md_ps = ps_md.tile([128, 512], f32, tag="md")
                    for p in range(4):
                        nc.tensor.matmul(
                            md_ps[:],
                            w1_sb[:, p * 8192 + ft * 256:
                                  p * 8192 + ft * 256 + 256]
                            .rearrange("q (k2 m) -> q k2 m", k2=2),
                            hT_c[:, p * 1024: p * 1024 + 1024]
                            .rearrange("q (k2 t) -> q k2 t", k2=2),
                            start=(p == 0), stop=(p == 3), perf_mode=DR)
                    nc.scalar.activation(
                        midT[:, ft * 512: ft * 512 + 512], md_ps[:],
                        AF.Relu, bias=b1_sb[:, ft: ft + 1])
                eo_sb = mck.tile([128, 4096], bf, tag="eo", bufs=2)
                for th in range(4):
                    for ec in range(2):
                        eo_ps = ps_eo.tile([128, 512], f32, tag="eo_ps")
                        for qq2 in range(16):
                            nc.tensor.matmul(
                                eo_ps[:],
                                midT[:, qq2 * 1024: qq2 * 1024 + 1024]
                                .rearrange("q (k2 t) -> q k2 t", k2=2)
                                [:, :, th * 128: th * 128 + 128],
                                w2_sb[:, qq2 * 2048: qq2 * 2048 + 2048]
                                .rearrange("q (k2 e) -> q k2 e", k2=2)
                                [:, :, ec * 512: ec * 512 + 512],
                                start=(qq2 == 0), stop=False, perf_mode=DR)
                        nc.tensor.matmul(
                            eo_ps[:], ones_row[:, 0:128],
                            b2_sb[:, ec * 512: ec * 512 + 512],
                            start=False, stop=True)
                        if ec == 0:
                            nc.scalar.activation(
                                eo_sb[:, th * 1024 + ec * 512:
                                      th * 1024 + ec * 512 + 512],
                                eo_ps[:], AF.Identity,
                                scale=pcol[:, th: th + 1])
                        else:
                            nc.vector.tensor_scalar_mul(
                                eo_sb[:, th * 1024 + ec * 512:
                                      th * 1024 + ec * 512 + 512],
                                eo_ps[:], pcol[:, th: th + 1])
                for th in range(4):
                    nc.sync.dma_start(
                        rs_ins[ck][th * 128: th * 128 + 128, :],
                        eo_sb[:, th * 1024: th * 1024 + 1024])
                nc.gpsimd.collective_compute(
                    "ReduceScatter", mybir.AluOpType.add,
                    replica_groups=[list(range(8))],
                    ins=[rs_ins[ck].opt()], outs=[rs_outs[ck].opt()])
                if ck >= 2:
                    do_fin(ck - 2)
            do_fin(6)
            do_fin(7)
        moe_cm.__exit__(None, None, None)

        cpool_cm.__exit__(None, None, None)
    nc.compile()
    return nc


def _prep_inputs(inputs):
    f = np.float32
    x = np.asarray(inputs["x"], f)
    wq, bq = np.asarray(inputs["wq"], f), np.asarray(inputs["bq"], f)
    wk, bk = np.asarray(inputs["wk"], f), np.asarray(inputs["bk"], f)
    wv, bv = np.asarray(inputs["wv"], f), np.asarray(inputs["bv"], f)
    wp, bp = np.asarray(inputs["wp"], f), np.asarray(inputs["bp"], f)
    ln1_g, ln1_b = np.asarray(inputs["ln1_g"], f), np.asarray(inputs["ln1_b"], f)
    ln2_g, ln2_b = np.asarray(inputs["ln2_g"], f), np.asarray(inputs["ln2_b"], f)
    wr, br = np.asarray(inputs["wr"], f), np.asarray(inputs["br"], f)
    w1, b1 = np.asarray(inputs["w1"], f), np.asarray(inputs["b1"], f)
    w2, b2 = np.asarray(inputs["w2"], f), np.asarray(inputs["b2"], f)

    def etile(a):  # [E, M] -> [128, 8*M]
        M = a.shape[1]
        return np.ascontiguousarray(
            a.reshape(8, 128, M).transpose(1, 0, 2).reshape(128, 8 * M))

    wq_f = wq.transpose(1, 0, 2).reshape(E, E)   # [e, h*64+d]
    wk_f = wk.transpose(1, 0, 2).reshape(E, E)
    wv_f = wv.transpose(1, 0, 2).reshape(E, E)
    # sect-major [k|v|q], each section e-tiled: contiguous section DMAs
    wqkv_t = np.concatenate(
        [etile(64.0 * wk_f), etile(64.0 * wv_f), etile(64.0 * wq_f)],
        axis=1).astype(F8)                                   # [128, 3*8192]
    bqk = 64.0 * np.concatenate(
        [bq.reshape(-1).reshape(8, 128).T,
         bk.reshape(-1).reshape(8, 128).T], axis=1).astype(f)
    wp_t = etile(64.0 * wp).astype(F8)                       # [128, 8*1024]
    w1e = (ln1_g[:, None] * w1).astype(f)                    # [n,E,FF]
    b1e = b1 + ln1_b @ w1                                    # [n,FF]
    wre = (ln1_g[:, None] * wr).astype(f)                    # [E,8]
    bre = br + ln1_b @ wr                                    # [8]
    ident = np.eye(128, dtype=BF16)

    in_maps = []
    for r in range(NCORE):
        b, p = r // 2, r % 2
        # interleaved token assignment: local s_loc <-> orig row 2*s_loc + p
        xs = np.ascontiguousarray(x[b, p::2, :])             # [512, E]
        xpart = np.ascontiguousarray(x[b, 1 - p::2, :])      # partner tokens
        xT_t = np.concatenate(
            [etile(np.ascontiguousarray(xs.T)),
             etile(np.ascontiguousarray(xpart.T))], axis=1).astype(F8)
        xr_t = np.ascontiguousarray(
            xs.reshape(4, 128, 1024).transpose(1, 0, 2).reshape(128, 4096), f)
        # final-phase x rows: group k = batch k%4, half A (k<4) or B;
        # rows [src-core-2b tokens | src-core-2b+1 tokens], this core's
        # 64-row slice starts at 64r
        R = 64 * r + np.arange(64)
        xr2_cols = []
        for k in range(8):
            b = k % 4
            local = R % 256 + (0 if k < 4 else 256)
            par = R // 256
            xr2_cols.append(x[b, 2 * local + par, :])
        xr2_t = np.ascontiguousarray(np.concatenate(xr2_cols, axis=1), f)
        # diagonal causal masks: half0 = own parity keys, half1 = partner
        masks = np.zeros((4, 128, 256), BF16)
        ti = np.arange(128)
        sj = np.arange(128)
        own = (ti[:, None] <= sj[None, :])
        part = (ti[:, None] <= sj[None, :]) if p == 1 else                (ti[:, None] < sj[None, :])
        for ss in range(4):
            masks[ss][:, 0:128] = own.astype(BF16)
            masks[ss][:, 128:256] = part.astype(BF16)
        perm = [r] + [i for i in range(NEXP) if i != r]
        # fp8 DoubleRow layouts; w1 x16, w2/wr x64 to clear e4m3 subnormals
        wr_p = etile(64.0 * wre[:, perm]).astype(F8)         # [128, 8*8]
        br_p = (64.0 * bre[perm]).reshape(1, 8).astype(BF16)
        w1_t = np.ascontiguousarray(
            (16.0 * w1e[r]).reshape(4, 2, 128, 32, 128)
            .transpose(2, 0, 3, 1, 4).reshape(128, 32768)).astype(F8)
        b1_t = np.ascontiguousarray(
            16.0 * b1e[r].reshape(32, 128).T, f)             # [128, 32]
        w2_t = np.ascontiguousarray(
            (64.0 * w2[r]).reshape(16, 2, 128, 1024)
            .transpose(2, 0, 1, 3).reshape(128, 32768)).astype(F8)
        in_maps.append({
            "xT": xT_t, "xr": xr_t, "xr2": xr2_t, "wqkv": wqkv_t, "bqk": bqk,
            "bv": (64.0 * bv).reshape(1, E).astype(BF16),
            "wp": wp_t, "bp": (64.0 * bp).reshape(1, E).astype(BF16),
            "masks": masks, "ident": ident,
            "w1": w1_t, "b1": b1_t, "w2": w2_t,
            "b2": (1024.0 * b2[r]).reshape(1, E).astype(BF16),
            "wr": wr_p, "br": br_p,
            "g2": np.broadcast_to(ln2_g, (128, E)).astype(f).copy(),
            "bl2": np.broadcast_to(ln2_b, (128, E)).astype(f).copy(),
        })
    return in_maps


def kernel(**inputs):
    from concourse import bass_utils
    if "nc" not in _CACHE:
        _CACHE["nc"] = _build_program()
    nc = _CACHE["nc"]
    in_maps = _prep_inputs(inputs)
    res = bass_utils.run_bass_kernel_spmd(
        nc, in_maps, core_ids=list(range(NCORE)))
    # group k = batch k%4, A-halves (k<4) or B-halves of the two src
    # cores; core r holds rows [64r, 64r+64) of each group
    full = np.empty((B, S, E), np.float32)
    for r in range(NCORE):
        o = res.results[r]["out"]                            # [8, 64, 1024]
        R = 64 * r + np.arange(64)
        for k in range(NCORE):
            b = k % 4
            local = R % 256 + (0 if k < 4 else 256)
            par = R // 256
            full[b, 2 * local + par, :] = o[k]
    return full



# revision 58
# speedup vs baseline: 1.0548x; 1.0105x over previous
"""Trainium2 Bass kernel for nn_Block_56616258896419 (moe_routing).

Self-contained: takes FULL inputs (as from setup_inputs()), returns FULL
[4,1024,1024] f32 output. Internally shards across 8 NeuronCores:
  - tokens 8-way (core r: batch r//2, sequence half r%2) for attention/LN
  - experts 8-way (core r computes expert r over ALL tokens) for the MoE
Collectives: pairwise AllGather of K/V, 8-way AllGather of LN1'd
activations (transposed, bf16), 4x chunked 8-way ReduceScatter of the
prob-weighted expert outputs (overlapped with MoE compute).
"""
import numpy as np
import ml_dtypes

B, S, E, H, HD, NEXP, FF = 4, 1024, 1024, 16, 64, 8, 4096
NCORE = 8
TOK = 512          # tokens per core
TC = 256           # MoE token-chunk
NCHUNK = (B * S) // TC
EPS = 1e-5
BF16 = ml_dtypes.bfloat16
F8 = ml_dtypes.float8_e4m3

_CACHE = {}


def _build_program():
    import concourse.bacc as bacc
    import concourse.mybir as mybir
    import concourse.tile as tile

    dt = mybir.dt
    f32, bf = dt.float32, dt.bfloat16
    AF = mybir.ActivationFunctionType
    ALU = mybir.AluOpType

    nc = bacc.Bacc("TRN2", target_bir_lowering=False, debug=False,
                   num_devices=NCORE)

    # ---------------- I/O ----------------
    def inp(name, shape, d):
        return nc.dram_tensor(name, shape, d, kind="ExternalInput").ap()

    fp8i = dt.float8e4
    xT_d = inp("xT", [128, 2 * 4096], fp8i)       # x^T [own|partner], e-tiled
    xr_d = inp("xr", [128, 4 * 1024], f32)        # x token-major, tt blocks
    xr2_d = inp("xr2", [64, 8 * 1024], f32)       # x rows per src-core group
    wqkv_d = inp("wqkv", [128, 8 * 3072], fp8i)   # [E,3E] e-tiled, x64
    bqk_d = inp("bqk", [128, 16], f32)            # x64
    bv_d = inp("bv", [1, 1024], bf)               # x64
    wp_d = inp("wp", [128, 8 * 1024], fp8i)       # x64
    bp_d = inp("bp", [1, 1024], bf)               # x64
    masks_d = inp("masks", [4, 128, 256], bf)     # diag-pair 0/1 masks
    ident_d = inp("ident", [128, 128], bf)
    fp8 = dt.float8e4
    w1_d = inp("w1", [128, 8 * 4096], fp8)        # ln1-folded, x16, DR pairs
    b1_d = inp("b1", [128, 32], f32)              # x16
    w2_d = inp("w2", [128, 32 * 1024], fp8)       # x64, DR pairs
    b2_d = inp("b2", [1, 1024], bf)               # x1024
    wr_d = inp("wr", [128, 8 * 8], fp8)           # ln1-folded, permuted, x64
    br_d = inp("br", [1, 8], bf)
    g2_d = inp("g2", [128, 1024], f32)            # ln2_g replicated
    bl2_d = inp("bl2", [128, 1024], f32)          # ln2_b replicated
    out_d = nc.dram_tensor("out", [8, 64, 1024], f32,
                           kind="ExternalOutput").ap()

    # ---------------- internal DRAM ----------------
    warm_in = nc.dram_tensor("warm_in", [1, 64], fp8).ap()
    warm_out = nc.dram_tensor("warm_out", [8, 1, 64], fp8,
                              addr_space="Shared").ap()
    hag_inA1 = nc.dram_tensor("hag_inA1", [8, 128, 128], fp8).ap()
    hag_inA2 = nc.dram_tensor("hag_inA2", [8, 128, 128], fp8).ap()
    hag_inB = nc.dram_tensor("hag_inB", [8, 128, 256], fp8).ap()
    hag_outA1 = nc.dram_tensor("hag_outA1", [8, 8, 128, 128], fp8,
                               addr_space="Shared").ap()
    hag_outA2 = nc.dram_tensor("hag_outA2", [8, 8, 128, 128], fp8,
                               addr_space="Shared").ap()
    hag_outB = nc.dram_tensor("hag_outB", [8, 8, 128, 256], fp8,
                              addr_space="Shared").ap()
    rs_ins = [nc.dram_tensor(f"rs_in{g}", [512, 1024], bf).ap()
              for g in range(8)]
    rs_outs = [nc.dram_tensor(f"rs_out{g}", [64, 1024], bf).ap()
               for g in range(8)]

    with tile.TileContext(nc) as tc:
        cpool_cm = tc.tile_pool(name="cpool", bufs=1, side="left")
        cpool = cpool_cm.__enter__()
        ones_row = cpool.tile([1, 128], bf)
        nc.vector.memset(ones_row[:], 1.0)
        ones_f = cpool.tile([1, 128], f32)
        nc.vector.memset(ones_f[:], 1.0)
        bqk_sb = cpool.tile([128, 16], f32)
        nc.sync.dma_start(bqk_sb[:], bqk_d[:])
        bv_sb = cpool.tile([1, 1024], bf)
        nc.sync.dma_start(bv_sb[:], bv_d[:])
        bp_sb = cpool.tile([1, 1024], bf)
        nc.sync.dma_start(bp_sb[:], bp_d[:])
        ident_sb = cpool.tile([128, 128], bf)
        nc.sync.dma_start(ident_sb[:], ident_d[:])
        wr_sb = cpool.tile([128, 64], fp8)
        nc.sync.dma_start(wr_sb[:], wr_d[:])
        br_sb = cpool.tile([1, 8], bf)
        nc.sync.dma_start(br_sb[:], br_d[:])
        b1_sb = cpool.tile([128, 32], f32)
        nc.sync.dma_start(b1_sb[:], b1_d[:])
        b2_sb = cpool.tile([1, 1024], bf)
        nc.sync.dma_start(b2_sb[:], b2_d[:])

        # ===== phase 1: local K/V for BOTH interleave-halves, then Q =====
        # all fp8 DoubleRow over e-tile pairs; weights x64, descaled on exit
        DR = mybir.MatmulPerfMode.DoubleRow
        qkv_cm = tc.tile_pool(name="qkv", bufs=1, side="right")
        qkv = qkv_cm.__enter__()
        xT_sb = qkv.tile([128, 8192], fp8)
        nc.sync.dma_start(xT_sb[:, 0:4096], xT_d[:, 0:4096])
        wqkv_sb = qkv.tile([128, 24576], fp8)
        # sect-major host layout [k|v|q]; spread across rings so the first
        # K matmul waits only ~1.5MB: K on scalar ring, V on gpsimd ring
        nc.scalar.dma_start(wqkv_sb[:, 0:8192], wqkv_d[:, 0:8192])
        nc.gpsimd.dma_start(wqkv_sb[:, 8192:16384], wqkv_d[:, 8192:16384])
        nc.sync.dma_start(xT_sb[:, 4096:8192], xT_d[:, 4096:8192])
        nc.sync.dma_start(wqkv_sb[:, 16384:24576], wqkv_d[:, 16384:24576])
        xT_v = xT_sb[:].rearrange("p (h e t) -> p h e t", h=2, e=8)
        wq_v = wqkv_sb[:].rearrange("p (s e c) -> p s e c", s=3, e=8)

        attn_cm = tc.tile_pool(name="attn", bufs=1, side="left")
        attn = attn_cm.__enter__()
        qT_sb = attn.tile([128, 4096], bf)
        kT_full = attn.tile([128, 8192], bf)     # [j][half*512 + s]
        v_full = attn.tile([128, 8192], bf)      # [u = half*4+tt][hd]
        # augmented V: per t-tile, 16 heads x (64 v-cols + 1 ones-col);
        # ones-cols set up front, v copied per u-tile inside the V loop
        v_aug = attn.tile([128, 8 * 1040], bf)
        for tt in range(8):
            nc.vector.memset(
                v_aug[:, tt * 1040: tt * 1040 + 1040]
                .rearrange("p (h dd) -> p h dd", dd=65)[:, :, 64:65], 1.0)

        with tc.tile_pool(name="ps_qkv", bufs=3, space="PSUM") as psq:
            for half in range(2):
                for j in range(8):
                    k_ps = psq.tile([128, 512], f32, tag="qk_ps")
                    for p in range(4):
                        nc.tensor.matmul(
                            k_ps[:],
                            wq_v[:, 0, 2 * p: 2 * p + 2,
                                 j * 128: j * 128 + 128],
                            xT_v[:, half, 2 * p: 2 * p + 2, :],
                            start=(p == 0), stop=(p == 3), perf_mode=DR)
                    nc.vector.tensor_scalar(
                        kT_full[:, j * 1024 + half * 512:
                                j * 1024 + half * 512 + 512], k_ps[:],
                        bqk_sb[:, 8 + j: 8 + j + 1], 1.0 / 64.0,
                        op0=ALU.add, op1=ALU.mult)
            # qT next (own tokens = half 0) so scores can start right away
            for j in range(8):
                q_ps = psq.tile([128, 512], f32, tag="qk_ps")
                for p in range(4):
                    nc.tensor.matmul(
                        q_ps[:],
                        wq_v[:, 2, 2 * p: 2 * p + 2, j * 128: j * 128 + 128],
                        xT_v[:, 0, 2 * p: 2 * p + 2, :],
                        start=(p == 0), stop=(p == 3), perf_mode=DR)
                nc.vector.tensor_scalar(
                    qT_sb[:, j * 512: j * 512 + 512], q_ps[:],
                    bqk_sb[:, j: j + 1], 0.125 / 64.0,
                    op0=ALU.add, op1=ALU.mult)
            for half in range(2):
                for tt in range(4):
                    u = half * 4 + tt
                    for c in range(2):
                        v_ps = psq.tile([128, 512], f32, tag="v_ps")
                        for p in range(4):
                            nc.tensor.matmul(
                                v_ps[:],
                                xT_v[:, half, 2 * p: 2 * p + 2,
                                     tt * 128: tt * 128 + 128],
                                wq_v[:, 1, 2 * p: 2 * p + 2,
                                     c * 512: c * 512 + 512],
                                start=(p == 0), stop=False, perf_mode=DR)
                        nc.tensor.matmul(
                            v_ps[:], ones_row[:, 0:128],
                            bv_sb[:, c * 512: c * 512 + 512],
                            start=False, stop=True)
                        nc.scalar.activation(
                            v_full[:, u * 1024 + c * 512:
                                   u * 1024 + c * 512 + 512], v_ps[:],
                            AF.Identity, scale=1.0 / 64.0)
                    nc.vector.tensor_copy(
                        v_aug[:, u * 1040: u * 1040 + 1040]
                        .rearrange("p (h dd) -> p h dd", dd=65)[:, :, 0:64],
                        v_full[:, u * 1024: u * 1024 + 1024]
                        .rearrange("p (h dd) -> p h dd", dd=64))
        qkv_cm.__exit__(None, None, None)

        # MoE weights prefetch on the vector ring (overlaps attention);
        # wp/x residual loads early on the sync ring so proj never waits
        moe_cm = tc.tile_pool(name="moe", bufs=1, side="right")
        moe = moe_cm.__enter__()
        w1_sb = moe.tile([128, 32768], fp8)
        w2_sb = moe.tile([128, 32768], fp8)
        for et in range(8):
            nc.gpsimd.dma_start(w1_sb[:, et * 4096: et * 4096 + 4096],
                                w1_d[:, et * 4096: et * 4096 + 4096])
        for ft8 in range(8):
            nc.gpsimd.dma_start(w2_sb[:, ft8 * 4096: ft8 * 4096 + 4096],
                                w2_d[:, ft8 * 4096: ft8 * 4096 + 4096])

        # warm the collective channel so the first real AllGather doesn't
        # pay rendezvous/ring-warmup latency on the critical path
        nc.gpsimd.collective_compute(
            "AllGather", mybir.AluOpType.bypass,
            replica_groups=[list(range(8))],
            ins=[warm_in.opt()], outs=[warm_out.opt()])

        # ============ phase 2: attention ============
        mask_sb = attn.tile([128, 1024], bf)
        for ss in range(4):
            nc.sync.dma_start(mask_sb[:, ss * 256: ss * 256 + 256],
                              masks_d[ss])

        proj_cm = tc.tile_pool(name="proj", bufs=1, side="right")
        projp = proj_cm.__enter__()
        catT_sb = projp.tile([128, 4096], bf)    # unnormalized heads^T
        catT8 = projp.tile([128, 4096], fp8)     # normalized, fp8 for proj
        wp_sb = projp.tile([128, 8192], fp8)
        nc.sync.dma_start(wp_sb[:], wp_d[:])
        x_sb = projp.tile([128, 4096], f32)
        nc.sync.dma_start(x_sb[:], xr_d[:])

        with tc.tile_pool(name="sc", bufs=4, side="left") as scp, \
             tc.tile_pool(name="ps_sc", bufs=2, space="PSUM") as ps_sc, \
             tc.tile_pool(name="ps_av", bufs=4, space="PSUM") as ps_av:
            # one N-restricted matmul per key-tile: keys (ph,tl) attend to
            # queries >= tl, so scores/exp/AV each run [128, (4-tl)*128].
            # PE emission is software-pipelined: scores run one key-tile
            # ahead of AV (covering exp latency), and each j's epilogue
            # (softmax-normalize via K=1 matmuls) is deferred into j+1.

            def epilogue(j, avps):
                # partition-shifting copies (0:64 -> 64:128, row 64 -> 0)
                # must run on ACT; lane-aligned ones go to DVE
                rcs = []
                for h01 in range(2):
                    if h01 == 0:
                        nc.vector.tensor_copy(
                            catT_sb[0:64, j * 512: j * 512 + 512],
                            avps[0][0:64, :])
                    else:
                        nc.scalar.copy(
                            catT_sb[64:128, j * 512: j * 512 + 512],
                            avps[1][0:64, :])
                    sm_f = scp.tile([1, 512], f32, tag="sm_f", bufs=4,
                                    name=f"smf{j}_{h01}")
                    nc.scalar.copy(sm_f[:], avps[h01][64:65, :])
                    rc_f = scp.tile([1, 512], f32, tag="rc_f", bufs=4,
                                    name=f"rcf{j}_{h01}")
                    nc.vector.reciprocal_approx_fast(rc_f[:], sm_f[:])
                    rc_h = scp.tile([1, 512], bf, tag="rc_h", bufs=4,
                                    name=f"rc{j}_{h01}")
                    nc.vector.tensor_copy(rc_h[:], rc_f[:])
                    rcs.append(rc_h)
                bc_ps = ps_sc.tile([128, 1024], f32, tag="sc",
                                   name=f"bc{j}")[:, 0:512]
                nc.tensor.matmul(bc_ps[0:64, :], ones_row[:, 0:64],
                                 rcs[0][:], start=True, stop=True)
                nc.tensor.matmul(bc_ps[64:128, :], ones_row[:, 0:64],
                                 rcs[1][:], start=True, stop=True)
                nc.vector.tensor_tensor(
                    catT_sb[:, j * 512: j * 512 + 512],
                    catT_sb[:, j * 512: j * 512 + 512], bc_ps[:],
                    op=ALU.mult)
                nc.vector.tensor_copy(catT8[:, j * 512: j * 512 + 512],
                                      catT_sb[:, j * 512: j * 512 + 512])

            # scores packed in (tl, tl+1) pairs -> one exp per pair; AV
            # runs one pair behind so exp latency is always covered
            pend = None
            for j in range(8):
                avps = [ps_av.tile([65, 512], f32, tag="av", name=f"av{j}_{k}")
                        for k in range(2)]
                for h01 in range(2):
                    po = 64 * h01
                    prev = None
                    first = True

                    def emit_av(pavs, start):
                        for pqo, pu, pexp, plo, pN in pavs:
                            nc.tensor.matmul(
                                avps[h01][:, pqo:512],
                                v_aug[:, pu * 1040 + (2 * j + h01) * 65:
                                      pu * 1040 + (2 * j + h01) * 65 + 65],
                                pexp[:, plo:plo + pN],
                                start=start, stop=False)
                            start = False

                    for ph in range(2):
                        for ta in (0, 2):
                            Na = (4 - ta) * 128
                            Nb = (4 - ta - 1) * 128
                            sc_ps = ps_sc.tile([128, 1024], f32, tag="sc",
                                               name=f"sc{j}_{h01}_{ph}_{ta}")
                            nc.tensor.matmul(
                                sc_ps[:, 0:Na],
                                kT_full[po:po + 64,
                                        j * 1024 + ph * 512 + ta * 128:
                                        j * 1024 + ph * 512 + ta * 128 + 128],
                                qT_sb[po:po + 64,
                                      j * 512 + ta * 128: j * 512 + 512],
                                start=True, stop=True)
                            nc.tensor.matmul(
                                sc_ps[:, Na:Na + Nb],
                                kT_full[po:po + 64,
                                        j * 1024 + ph * 512 + ta * 128 + 128:
                                        j * 1024 + ph * 512 + ta * 128 + 256],
                                qT_sb[po:po + 64,
                                      j * 512 + ta * 128 + 128: j * 512 + 512],
                                start=True, stop=True)
                            expT = scp.tile([128, 1024], bf, tag="expT",
                                            name=f"ex{j}_{h01}_{ph}_{ta}")
                            nc.scalar.activation(
                                expT[:, 0:Na], sc_ps[:, 0:Na], AF.Exp)
                            nc.scalar.activation(
                                expT[:, Na:Na + Nb], sc_ps[:, Na:Na + Nb],
                                AF.Exp)
                            nc.vector.tensor_tensor(
                                expT[:, 0:128], expT[:, 0:128],
                                mask_sb[:, ta * 256 + ph * 128:
                                        ta * 256 + ph * 128 + 128],
                                op=ALU.mult)
                            nc.vector.tensor_tensor(
                                expT[:, Na:Na + 128], expT[:, Na:Na + 128],
                                mask_sb[:, (ta + 1) * 256 + ph * 128:
                                        (ta + 1) * 256 + ph * 128 + 128],
                                op=ALU.mult)
                            if prev is not None:
                                emit_av(prev, first)
                                first = False
                            if pend is not None and h01 == 0 and \
                                    ph == 1 and ta == 0:
                                epilogue(*pend)
                                pend = None
                            u = ph * 4 + ta
                            prev = [(ta * 128, u, expT, 0, Na),
                                    ((ta + 1) * 128, u + 1, expT, Na, Nb)]
                    emit_av(prev[:1], first)
                    pqo, pu, pexp, plo, pN = prev[1]
                    nc.tensor.matmul(
                        avps[h01][:, pqo:512],
                        v_aug[:, pu * 1040 + (2 * j + h01) * 65:
                              pu * 1040 + (2 * j + h01) * 65 + 65],
                        pexp[:, plo:plo + pN],
                        start=False, stop=True)
                pend = (j, avps)
            epilogue(*pend)
        attn_cm.__exit__(None, None, None)

        h_sb = projp.tile([128, 4096], bf)
        hT_stage = projp.tile([128, 4096], fp8)
        cat_v = catT8[:].rearrange("p (j t) -> p j t", j=8)
        wp_v = wp_sb[:].rearrange("p (j e) -> p j e", j=8)

        with tc.tile_pool(name="prw", bufs=2, side="left") as prp, \
             tc.tile_pool(name="ps_pr", bufs=4, space="PSUM") as ps_pr:
            for tt in range(4):
                y_sb = prp.tile([128, 1024], f32, tag="y")
                for ec in range(2):
                    ao_ps = ps_pr.tile([128, 512], f32, tag="ao")
                    for p in range(4):
                        nc.tensor.matmul(
                            ao_ps[:],
                            cat_v[:, 2 * p: 2 * p + 2,
                                  tt * 128: tt * 128 + 128],
                            wp_v[:, 2 * p: 2 * p + 2,
                                 ec * 512: ec * 512 + 512],
                            start=(p == 0), stop=False, perf_mode=DR)
                    nc.tensor.matmul(
                        ao_ps[:], ones_row[:, 0:128],
                        bp_sb[:, ec * 512: ec * 512 + 512],
                        start=False, stop=True)
                    nc.vector.tensor_scalar(
                        y_sb[:, ec * 512: ec * 512 + 512], ao_ps[:],
                        1.0 / 64.0, None, op0=ALU.mult)
                    nc.vector.tensor_tensor(
                        y_sb[:, ec * 512: ec * 512 + 512],
                        y_sb[:, ec * 512: ec * 512 + 512],
                        x_sb[:, tt * 1024 + ec * 512: tt * 1024 + ec * 512 + 512],
                        op=ALU.add)
                # LN1 stats
                mean = prp.tile([128, 1], f32, tag="mean")
                nc.vector.reduce_sum(mean[:], y_sb[:], axis=mybir.AxisListType.X)
                nc.vector.tensor_scalar_mul(mean[:], mean[:], 1.0 / 1024.0)
                sq = prp.tile([128, 1024], bf, tag="sq")
                sqs = prp.tile([128, 1], f32, tag="sqs")
                nc.scalar.activation(sq[:], y_sb[:], AF.Square,
                                     accum_out=sqs[:])
                m2 = prp.tile([128, 1], f32, tag="m2")
                nc.scalar.activation(m2[:], mean[:], AF.Square)
                var = prp.tile([128, 1], f32, tag="var")
                nc.vector.tensor_scalar(var[:], sqs[:], 1.0 / 1024.0, EPS,
                                        op0=ALU.mult, op1=ALU.add)
                nc.vector.tensor_tensor(var[:], var[:], m2[:], op=ALU.subtract)
                std = prp.tile([128, 1], f32, tag="std")
                nc.scalar.activation(std[:], var[:], AF.Sqrt)
                rstd = prp.tile([128, 1], f32, tag="rstd")
                nc.vector.reciprocal(rstd[:], std[:])
                nc.vector.tensor_scalar(
                    h_sb[:, tt * 1024: tt * 1024 + 1024], y_sb[:],
                    mean[:], rstd[:], op0=ALU.subtract, op1=ALU.mult)
                # transpose h tile -> hT
                for et in range(8):
                    tp = ps_pr.tile([128, 128], bf, tag="tp")
                    nc.tensor.transpose(
                        tp[:], h_sb[:, tt * 1024 + et * 128:
                                    tt * 1024 + et * 128 + 128], ident_sb[:])
                    nc.scalar.copy(
                        hT_stage[:, et * 512 + tt * 128:
                                 et * 512 + tt * 128 + 128], tp[:])
                if tt == 0:
                    for et in range(8):
                        nc.gpsimd.dma_start(
                            hag_inA1[et],
                            hT_stage[:, et * 512: et * 512 + 128])
                    nc.gpsimd.collective_compute(
                        "AllGather", mybir.AluOpType.bypass,
                        replica_groups=[list(range(8))],
                        ins=[hag_inA1.opt()], outs=[hag_outA1.opt()])
                if tt == 1:
                    for et in range(8):
                        nc.gpsimd.dma_start(
                            hag_inA2[et],
                            hT_stage[:, et * 512 + 128: et * 512 + 256])
                    nc.gpsimd.collective_compute(
                        "AllGather", mybir.AluOpType.bypass,
                        replica_groups=[list(range(8))],
                        ins=[hag_inA2.opt()], outs=[hag_outA2.opt()])
                if tt == 3:
                    for et in range(8):
                        nc.gpsimd.dma_start(
                            hag_inB[et],
                            hT_stage[:, et * 512 + 256: et * 512 + 512])
                    nc.gpsimd.collective_compute(
                        "AllGather", mybir.AluOpType.bypass,
                        replica_groups=[list(range(8))],
                        ins=[hag_inB.opt()], outs=[hag_outB.opt()])
            # (b) PE warm-keepers over the h-AG wait
            for wi in range(16):
                wm_ps = ps_pr.tile([128, 512], f32, tag="ao", name=f"wm{wi}")
                nc.tensor.matmul(wm_ps[:], catT8[:, 0:128],
                                 wp_sb[:, 0:512], start=True, stop=True)
        proj_cm.__exit__(None, None, None)

        # ============ phase 3: MoE (expert r over all tokens) ============
        # fin (residual+LN2) work is interleaved per completed RS group so
        # only the last group's LN2 sits on the tail
        with tc.tile_pool(name="fin", bufs=2, side="left") as fin, \
             tc.tile_pool(name="mchunk", bufs=2, side="left") as mck, \
             tc.tile_pool(name="ps_md", bufs=2, space="PSUM") as ps_md, \
             tc.tile_pool(name="ps_eo", bufs=3, space="PSUM") as ps_eo:
            x2_sb = fin.tile([64, 8192], f32, bufs=1)
            nc.sync.dma_start(x2_sb[:], xr2_d[:])
            g2_sb = fin.tile([128, 1024], f32, bufs=1)
            nc.sync.dma_start(g2_sb[:], g2_d[:])
            bl2_sb = fin.tile([128, 1024], f32, bufs=1)
            nc.sync.dma_start(bl2_sb[:], bl2_d[:])

            def do_fin(g):
                y2b = fin.tile([64, 1024], bf, tag="y2b", name=f"y2b{g}")
                nc.sync.dma_start(y2b[:], rs_outs[g][:])
                y2 = fin.tile([64, 1024], f32, tag="y2", name=f"y2{g}")
                nc.vector.tensor_tensor(
                    y2[:], y2b[:], x2_sb[:, g * 1024: g * 1024 + 1024],
                    op=ALU.add)
                mean = fin.tile([64, 1], f32, tag="mean2")
                nc.vector.reduce_sum(mean[:], y2[:], axis=mybir.AxisListType.X)
                nc.vector.tensor_scalar_mul(mean[:], mean[:], 1.0 / 1024.0)
                sq = fin.tile([64, 1024], f32, tag="sq2")
                sqs = fin.tile([64, 1], f32, tag="sqs2")
                nc.scalar.activation(sq[:], y2[:], AF.Square, accum_out=sqs[:])
                m2 = fin.tile([64, 1], f32, tag="m22")
                nc.scalar.activation(m2[:], mean[:], AF.Square)
                var = fin.tile([64, 1], f32, tag="var2")
                nc.vector.tensor_scalar(var[:], sqs[:], 1.0 / 1024.0, EPS,
                                        op0=ALU.mult, op1=ALU.add)
                nc.vector.tensor_tensor(var[:], var[:], m2[:],
                                        op=ALU.subtract)
                std = fin.tile([64, 1], f32, tag="std2")
                nc.scalar.activation(std[:], var[:], AF.Sqrt)
                rstd = fin.tile([64, 1], f32, tag="rstd2")
                nc.vector.reciprocal(rstd[:], std[:])
                on = fin.tile([64, 1024], f32, tag="on", name=f"on{g}")
                nc.vector.tensor_scalar(on[:], y2[:], mean[:], rstd[:],
                                        op0=ALU.subtract, op1=ALU.mult)
                nc.vector.tensor_tensor(on[:], on[:], g2_sb[0:64, :],
                                        op=ALU.mult)
                nc.vector.tensor_tensor(on[:], on[:], bl2_sb[0:64, :],
                                        op=ALU.add)
                nc.sync.dma_start(out_d[g], on[:])
            DR = mybir.MatmulPerfMode.DoubleRow
            # 512-token chunks: ck<4 pairs two cores' A-halves (available
            # right after the early AllGathers), ck>=4 pairs their B-halves
            for ck in range(8):
                hT_c = mck.tile([128, 4096], fp8, tag="hT_c")
                if ck < 4:
                    s0, s1 = 2 * ck, 2 * ck + 1
                    for et in range(8):
                        nc.gpsimd.dma_start(
                            hT_c[:, et * 512: et * 512 + 128],
                            hag_outA1[s0, et])
                        nc.gpsimd.dma_start(
                            hT_c[:, et * 512 + 128: et * 512 + 256],
                            hag_outA2[s0, et])
                        nc.gpsimd.dma_start(
                            hT_c[:, et * 512 + 256: et * 512 + 384],
                            hag_outA1[s1, et])
                        nc.gpsimd.dma_start(
                            hT_c[:, et * 512 + 384: et * 512 + 512],
                            hag_outA2[s1, et])
                else:
                    s0, s1 = 2 * (ck - 4), 2 * (ck - 4) + 1
                    for et in range(8):
                        nc.gpsimd.dma_start(
                            hT_c[:, et * 512: et * 512 + 256],
                            hag_outB[s0, et])
                        nc.gpsimd.dma_start(
                            hT_c[:, et * 512 + 256: et * 512 + 512],
                            hag_outB[s1, et])
                pcol = mck.tile([128, 4], f32, tag="pcol")
                for th in range(4):
                    lg_ps = ps_eo.tile([128, 8], f32, tag="lg")
                    for p in range(4):
                        nc.tensor.matmul(
                            lg_ps[:],
                            hT_c[:, p * 1024: p * 1024 + 1024]
                            .rearrange("q (k2 t) -> q k2 t", k2=2)
                            [:, :, th * 128: th * 128 + 128],
                            wr_sb[:, p * 16: p * 16 + 16]
                            .rearrange("q (k2 n) -> q k2 n", k2=2),
                            start=(p == 0), stop=False, perf_mode=DR)
                    nc.tensor.matmul(lg_ps[:], ones_row[:, 0:128], br_sb[:],
                                     start=False, stop=True)
                    pe = mck.tile([128, 8], f32, tag="pe")
                    ps = mck.tile([128, 1], f32, tag="ps")
                    nc.scalar.activation(pe[:], lg_ps[:], AF.Exp,
                                         scale=1.0 / 64.0, accum_out=ps[:])
                    ps2 = mck.tile([128, 1], f32, tag="ps2")
                    nc.vector.tensor_scalar_mul(ps2[:], ps[:], 1024.0)
                    pr = mck.tile([128, 1], f32, tag="pr")
                    nc.vector.reciprocal(pr[:], ps2[:])
                    nc.vector.tensor_tensor(pcol[:, th:th + 1], pe[:, 0:1],
                                            pr[:], op=ALU.mult)
                midT = mck.tile([128, 16384], fp8, tag="midT", bufs=2)
                for ft in range(32):
                    md_ps = ps_md.tile([128, 512], f32, tag="md")
                    for p in range(4):
                        nc.tensor.matmul(
                            md_ps[:],
                            w1_sb[:, p * 8192 + ft * 256:
                                  p * 8192 + ft * 256 + 256]
                            .rearrange("q (k2 m) -> q k2 m", k2=2),
                            hT_c[:, p * 1024: p * 1024 + 1024]
                            .rearrange("q (k2 t) -> q k2 t", k2=2),
                            start=(p == 0), stop=(p == 3), perf_mode=DR)
                    nc.scalar.activation(
                        midT[:, ft * 512: ft * 512 + 512], md_ps[:],
                        AF.Relu, bias=b1_sb[:, ft: ft + 1])
                eo_sb = mck.tile([128, 4096], bf, tag="eo", bufs=2)
                for th in range(4):
                    for ec in range(2):
                        eo_ps = ps_eo.tile([128, 512], f32, tag="eo_ps")
                        for qq2 in range(16):
                            nc.tensor.matmul(
                                eo_ps[:],
                                midT[:, qq2 * 1024: qq2 * 1024 + 1024]
                                .rearrange("q (k2 t) -> q k2 t", k2=2)
                                [:, :, th * 128: th * 128 + 128],
                                w2_sb[:, qq2 * 2048: qq2 * 2048 + 2048]
                                .rearrange("q (k2 e) -> q k2 e", k2=2)
                                [:, :, ec * 512: ec * 512 + 512],
                                start=(qq2 == 0), stop=False, perf_mode=DR)
                        nc.tensor.matmul(
                            eo_ps[:], ones_row[:, 0:128],
                            b2_sb[:, ec * 512: ec * 512 + 512],
                            start=False, stop=True)
                        if ec == 0:
                            nc.scalar.activation(
                                eo_sb[:, th * 1024 + ec * 512:
                                      th * 1024 + ec * 512 + 512],
                                eo_ps[:], AF.Identity,
                                scale=pcol[:, th: th + 1])
                        else:
                            nc.vector.tensor_scalar_mul(
                                eo_sb[:, th * 1024 + ec * 512:
                                      th * 1024 + ec * 512 + 512],
                                eo_ps[:], pcol[:, th: th + 1])
                for th in range(4):
                    nc.sync.dma_start(
                        rs_ins[ck][th * 128: th * 128 + 128, :],
                        eo_sb[:, th * 1024: th * 1024 + 1024])
                nc.gpsimd.collective_compute(
                    "ReduceScatter", mybir.AluOpType.add,
                    replica_groups=[list(range(8))],
                    ins=[rs_ins[ck].opt()], outs=[rs_outs[ck].opt()])
                if ck >= 2:
                    do_fin(ck - 2)
            do_fin(6)
            do_fin(7)
        moe_cm.__exit__(None, None, None)

        cpool_cm.__exit__(None, None, None)
    nc.compile()
    return nc


def _prep_inputs(inputs):
    f = np.float32
    x = np.asarray(inputs["x"], f)
    wq, bq = np.asarray(inputs["wq"], f), np.asarray(inputs["bq"], f)
    wk, bk = np.asarray(inputs["wk"], f), np.asarray(inputs["bk"], f)
    wv, bv = np.asarray(inputs["wv"], f), np.asarray(inputs["bv"], f)
    wp, bp = np.asarray(inputs["wp"], f), np.asarray(inputs["bp"], f)
    ln1_g, ln1_b = np.asarray(inputs["ln1_g"], f), np.asarray(inputs["ln1_b"], f)
    ln2_g, ln2_b = np.asarray(inputs["ln2_g"], f), np.asarray(inputs["ln2_b"], f)
    wr, br = np.asarray(inputs["wr"], f), np.asarray(inputs["br"], f)
    w1, b1 = np.asarray(inputs["w1"], f), np.asarray(inputs["b1"], f)
    w2, b2 = np.asarray(inputs["w2"], f), np.asarray(inputs["b2"], f)

    def etile(a):  # [E, M] -> [128, 8*M]
        M = a.shape[1]
        return np.ascontiguousarray(
            a.reshape(8, 128, M).transpose(1, 0, 2).reshape(128, 8 * M))

    wq_f = wq.transpose(1, 0, 2).reshape(E, E)   # [e, h*64+d]
    wk_f = wk.transpose(1, 0, 2).reshape(E, E)
    wv_f = wv.transpose(1, 0, 2).reshape(E, E)
    # sect-major [k|v|q], each section e-tiled: contiguous section DMAs
    wqkv_t = np.concatenate(
        [etile(64.0 * wk_f), etile(64.0 * wv_f), etile(64.0 * wq_f)],
        axis=1).astype(F8)                                   # [128, 3*8192]
    bqk = 64.0 * np.concatenate(
        [bq.reshape(-1).reshape(8, 128).T,
         bk.reshape(-1).reshape(8, 128).T], axis=1).astype(f)
    wp_t = etile(64.0 * wp).astype(F8)                       # [128, 8*1024]
    w1e = (ln1_g[:, None] * w1).astype(f)                    # [n,E,FF]
    b1e = b1 + ln1_b @ w1                                    # [n,FF]
    wre = (ln1_g[:, None] * wr).astype(f)                    # [E,8]
    bre = br + ln1_b @ wr                                    # [8]
    ident = np.eye(128, dtype=BF16)

    in_maps = []
    for r in range(NCORE):
        b, p = r // 2, r % 2
        # interleaved token assignment: local s_loc <-> orig row 2*s_loc + p
        xs = np.ascontiguousarray(x[b, p::2, :])             # [512, E]
        xpart = np.ascontiguousarray(x[b, 1 - p::2, :])      # partner tokens
        xT_t = np.concatenate(
            [etile(np.ascontiguousarray(xs.T)),
             etile(np.ascontiguousarray(xpart.T))], axis=1).astype(F8)
        xr_t = np.ascontiguousarray(
            xs.reshape(4, 128, 1024).transpose(1, 0, 2).reshape(128, 4096), f)
        # final-phase x rows: group k = batch k%4, half A (k<4) or B;
        # rows [src-core-2b tokens | src-core-2b+1 tokens], this core's
        # 64-row slice starts at 64r
        R = 64 * r + np.arange(64)
        xr2_cols = []
        for k in range(8):
            b = k % 4
            local = R % 256 + (0 if k < 4 else 256)
            par = R // 256
            xr2_cols.append(x[b, 2 * local + par, :])
        xr2_t = np.ascontiguousarray(np.concatenate(xr2_cols, axis=1), f)
        # diagonal causal masks: half0 = own parity keys, half1 = partner
        masks = np.zeros((4, 128, 256), BF16)
        ti = np.arange(128)
        sj = np.arange(128)
        own = (ti[:, None] <= sj[None, :])
        part = (ti[:, None] <= sj[None, :]) if p == 1 else                (ti[:, None] < sj[None, :])
        for ss in range(4):
            masks[ss][:, 0:128] = own.astype(BF16)
            masks[ss][:, 128:256] = part.astype(BF16)
        perm = [r] + [i for i in range(NEXP) if i != r]
        # fp8 DoubleRow layouts; w1 x16, w2/wr x64 to clear e4m3 subnormals
        wr_p = etile(64.0 * wre[:, perm]).astype(F8)         # [128, 8*8]
        br_p = (64.0 * bre[perm]).reshape(1, 8).astype(BF16)
        w1_t = np.ascontiguousarray(
            (16.0 * w1e[r]).reshape(4, 2, 128, 32, 128)
            .transpose(2, 0, 3, 1, 4).reshape(128, 32768)).astype(F8)
        b1_t = np.ascontiguousarray(
            16.0 * b1e[r].reshape(32, 128).T, f)             # [128, 32]
        w2_t = np.ascontiguousarray(
            (64.0 * w2[r]).reshape(16, 2, 128, 1024)
            .transpose(2, 0, 1, 3).reshape(128, 32768)).astype(F8)
        in_maps.append({
            "xT": xT_t, "xr": xr_t, "xr2": xr2_t, "wqkv": wqkv_t, "bqk": bqk,
            "bv": (64.0 * bv).reshape(1, E).astype(BF16),
            "wp": wp_t, "bp": (64.0 * bp).reshape(1, E).astype(BF16),
            "masks": masks, "ident": ident,
            "w1": w1_t, "b1": b1_t, "w2": w2_t,
            "b2": (1024.0 * b2[r]).reshape(1, E).astype(BF16),
            "wr": wr_p, "br": br_p,
            "g2": np.broadcast_to(ln2_g, (128, E)).astype(f).copy(),
            "bl2": np.broadcast_to(ln2_b, (128, E)).astype(f).copy(),
        })
    return in_maps


def kernel(**inputs):
    from concourse import bass_utils
    if "nc" not in _CACHE:
        _CACHE["nc"] = _build_program()
    nc = _CACHE["nc"]
    in_maps = _prep_inputs(inputs)
    res = bass_utils.run_bass_kernel_spmd(
        nc, in_maps, core_ids=list(range(NCORE)))
    # group k = batch k%4, A-halves (k<4) or B-halves of the two src
    # cores; core r holds rows [64r, 64r+64) of each group
    full = np.empty((B, S, E), np.float32)
    for r in range(NCORE):
        o = res.results[r]["out"]                            # [8, 64, 1024]
        R = 64 * r + np.arange(64)
        for k in range(NCORE):
            b = k % 4
            local = R % 256 + (0 if k < 4 else 256)
            par = R // 256
            full[b, 2 * local + par, :] = o[k]
    return full



# revision 65
# speedup vs baseline: 1.0744x; 1.0186x over previous
"""Trainium2 Bass kernel for nn_Block_56616258896419 (moe_routing).

Self-contained: takes FULL inputs (as from setup_inputs()), returns FULL
[4,1024,1024] f32 output. Internally shards across 8 NeuronCores:
  - tokens 8-way (core r: batch r//2, sequence half r%2) for attention/LN
  - experts 8-way (core r computes expert r over ALL tokens) for the MoE
Collectives: pairwise AllGather of K/V, 8-way AllGather of LN1'd
activations (transposed, bf16), 4x chunked 8-way ReduceScatter of the
prob-weighted expert outputs (overlapped with MoE compute).
"""
import numpy as np
import ml_dtypes

B, S, E, H, HD, NEXP, FF = 4, 1024, 1024, 16, 64, 8, 4096
NCORE = 8
TOK = 512          # tokens per core
TC = 256           # MoE token-chunk
NCHUNK = (B * S) // TC
EPS = 1e-5
BF16 = ml_dtypes.bfloat16
F8 = ml_dtypes.float8_e4m3

_CACHE = {}


def _build_program():
    import concourse.bacc as bacc
    import concourse.mybir as mybir
    import concourse.tile as tile

    dt = mybir.dt
    f32, bf = dt.float32, dt.bfloat16
    AF = mybir.ActivationFunctionType
    ALU = mybir.AluOpType

    nc = bacc.Bacc("TRN2", target_bir_lowering=False, debug=False,
                   num_devices=NCORE)

    # ---------------- I/O ----------------
    def inp(name, shape, d):
        return nc.dram_tensor(name, shape, d, kind="ExternalInput").ap()

    fp8i = dt.float8e4
    xT_d = inp("xT", [128, 2 * 4096], fp8i)       # x^T [own|partner], e-tiled
    xr_d = inp("xr", [128, 4 * 1024], f32)        # x token-major, tt blocks
    xr2_d = inp("xr2", [64, 8 * 1024], f32)       # x rows per src-core group
    wqkv_d = inp("wqkv", [128, 8 * 3072], fp8i)   # [E,3E] e-tiled, x64
    bqk_d = inp("bqk", [128, 16], f32)            # x64
    bv_d = inp("bv", [1, 1024], bf)               # x64
    wp_d = inp("wp", [128, 8 * 1024], fp8i)       # x64
    bp_d = inp("bp", [1, 1024], bf)               # x64
    masks_d = inp("masks", [4, 128, 256], bf)     # diag-pair 0/1 masks
    ident_d = inp("ident", [128, 128], bf)
    fp8 = dt.float8e4
    w1_d = inp("w1", [128, 8 * 4096], fp8)        # ln1-folded, x16, DR pairs
    b1_d = inp("b1", [128, 32], f32)              # x16
    w2_d = inp("w2", [128, 32 * 1024], fp8)       # x64, DR pairs
    b2_d = inp("b2", [1, 1024], bf)               # x1024
    wr_d = inp("wr", [128, 8 * 8], fp8)           # ln1-folded, permuted, x64
    br_d = inp("br", [1, 8], bf)
    g2_d = inp("g2", [128, 1024], f32)            # ln2_g replicated
    bl2_d = inp("bl2", [128, 1024], f32)          # ln2_b replicated
    out_d = nc.dram_tensor("out", [8, 64, 1024], f32,
                           kind="ExternalOutput").ap()

    # ---------------- internal DRAM ----------------
    warm_in = nc.dram_tensor("warm_in", [1, 64], fp8).ap()
    warm_out = nc.dram_tensor("warm_out", [8, 1, 64], fp8,
                              addr_space="Shared").ap()
    warm2_in = nc.dram_tensor("warm2_in", [1, 64], fp8).ap()
    warm2_out = nc.dram_tensor("warm2_out", [8, 1, 64], fp8,
                               addr_space="Shared").ap()
    hag_inA1 = nc.dram_tensor("hag_inA1", [8, 128, 128], fp8).ap()
    hag_inA2 = nc.dram_tensor("hag_inA2", [8, 128, 128], fp8).ap()
    hag_inB = nc.dram_tensor("hag_inB", [8, 128, 256], fp8).ap()
    hag_outA1 = nc.dram_tensor("hag_outA1", [8, 8, 128, 128], fp8,
                               addr_space="Shared").ap()
    hag_outA2 = nc.dram_tensor("hag_outA2", [8, 8, 128, 128], fp8,
                               addr_space="Shared").ap()
    hag_outB = nc.dram_tensor("hag_outB", [8, 8, 128, 256], fp8,
                              addr_space="Shared").ap()
    rs_ins = [nc.dram_tensor(f"rs_in{g}", [512, 1024], bf).ap()
              for g in range(8)]
    rs_outs = [nc.dram_tensor(f"rs_out{g}", [64, 1024], bf).ap()
               for g in range(8)]

    with tile.TileContext(nc) as tc:
        cpool_cm = tc.tile_pool(name="cpool", bufs=1, side="left")
        cpool = cpool_cm.__enter__()
        ones_row = cpool.tile([1, 128], bf)
        nc.vector.memset(ones_row[:], 1.0)
        ones_f = cpool.tile([1, 128], f32)
        nc.vector.memset(ones_f[:], 1.0)
        bqk_sb = cpool.tile([128, 16], f32)
        nc.sync.dma_start(bqk_sb[:], bqk_d[:])
        bv_sb = cpool.tile([1, 1024], bf)
        nc.sync.dma_start(bv_sb[:], bv_d[:])
        bp_sb = cpool.tile([1, 1024], bf)
        nc.sync.dma_start(bp_sb[:], bp_d[:])
        ident_sb = cpool.tile([128, 128], bf)
        nc.sync.dma_start(ident_sb[:], ident_d[:])
        wr_sb = cpool.tile([128, 64], fp8)
        nc.sync.dma_start(wr_sb[:], wr_d[:])
        br_sb = cpool.tile([1, 8], bf)
        nc.sync.dma_start(br_sb[:], br_d[:])
        b1_sb = cpool.tile([128, 32], f32)
        nc.sync.dma_start(b1_sb[:], b1_d[:])
        b2_sb = cpool.tile([1, 1024], bf)
        nc.sync.dma_start(b2_sb[:], b2_d[:])

        # ===== phase 1: local K/V for BOTH interleave-halves, then Q =====
        # all fp8 DoubleRow over e-tile pairs; weights x64, descaled on exit
        DR = mybir.MatmulPerfMode.DoubleRow
        qkv_cm = tc.tile_pool(name="qkv", bufs=1, side="right")
        qkv = qkv_cm.__enter__()
        xT_sb = qkv.tile([128, 8192], fp8)
        nc.sync.dma_start(xT_sb[:, 0:4096], xT_d[:, 0:4096])
        wqkv_sb = qkv.tile([128, 24576], fp8)
        # sect-major host layout [k|v|q]; spread across rings so the first
        # K matmul waits only ~1.5MB: K on scalar ring, V on gpsimd ring
        nc.scalar.dma_start(wqkv_sb[:, 0:8192], wqkv_d[:, 0:8192])
        nc.gpsimd.dma_start(wqkv_sb[:, 8192:16384], wqkv_d[:, 8192:16384])
        nc.sync.dma_start(xT_sb[:, 4096:8192], xT_d[:, 4096:8192])
        nc.sync.dma_start(wqkv_sb[:, 16384:24576], wqkv_d[:, 16384:24576])
        xT_v = xT_sb[:].rearrange("p (h e t) -> p h e t", h=2, e=8)
        wq_v = wqkv_sb[:].rearrange("p (s e c) -> p s e c", s=3, e=8)

        attn_cm = tc.tile_pool(name="attn", bufs=1, side="left")
        attn = attn_cm.__enter__()
        qT_sb = attn.tile([128, 4096], bf)
        kT_full = attn.tile([128, 8192], bf)     # [j][half*512 + s]
        v_full = attn.tile([128, 8192], bf)      # [u = half*4+tt][hd]
        # augmented V: per t-tile, 16 heads x (64 v-cols + 1 ones-col);
        # ones-cols set up front, v copied per u-tile inside the V loop
        v_aug = attn.tile([128, 8 * 1040], bf)
        for tt in range(8):
            nc.vector.memset(
                v_aug[:, tt * 1040: tt * 1040 + 1040]
                .rearrange("p (h dd) -> p h dd", dd=65)[:, :, 64:65], 1.0)

        with tc.tile_pool(name="ps_qkv", bufs=3, space="PSUM") as psq:
            for half in range(2):
                for j in range(8):
                    k_ps = psq.tile([128, 512], f32, tag="qk_ps")
                    for p in range(4):
                        nc.tensor.matmul(
                            k_ps[:],
                            wq_v[:, 0, 2 * p: 2 * p + 2,
                                 j * 128: j * 128 + 128],
                            xT_v[:, half, 2 * p: 2 * p + 2, :],
                            start=(p == 0), stop=(p == 3), perf_mode=DR)
                    nc.vector.tensor_scalar(
                        kT_full[:, j * 1024 + half * 512:
                                j * 1024 + half * 512 + 512], k_ps[:],
                        bqk_sb[:, 8 + j: 8 + j + 1], 1.0 / 64.0,
                        op0=ALU.add, op1=ALU.mult)
            # qT next (own tokens = half 0) so scores can start right away
            for j in range(8):
                q_ps = psq.tile([128, 512], f32, tag="qk_ps")
                for p in range(4):
                    nc.tensor.matmul(
                        q_ps[:],
                        wq_v[:, 2, 2 * p: 2 * p + 2, j * 128: j * 128 + 128],
                        xT_v[:, 0, 2 * p: 2 * p + 2, :],
                        start=(p == 0), stop=(p == 3), perf_mode=DR)
                nc.vector.tensor_scalar(
                    qT_sb[:, j * 512: j * 512 + 512], q_ps[:],
                    bqk_sb[:, j: j + 1], 0.125 / 64.0,
                    op0=ALU.add, op1=ALU.mult)
            for half in range(2):
                for tt in range(4):
                    u = half * 4 + tt
                    for c in range(2):
                        v_ps = psq.tile([128, 512], f32, tag="v_ps")
                        for p in range(4):
                            nc.tensor.matmul(
                                v_ps[:],
                                xT_v[:, half, 2 * p: 2 * p + 2,
                                     tt * 128: tt * 128 + 128],
                                wq_v[:, 1, 2 * p: 2 * p + 2,
                                     c * 512: c * 512 + 512],
                                start=(p == 0), stop=False, perf_mode=DR)
                        nc.tensor.matmul(
                            v_ps[:], ones_row[:, 0:128],
                            bv_sb[:, c * 512: c * 512 + 512],
                            start=False, stop=True)
                        nc.scalar.activation(
                            v_full[:, u * 1024 + c * 512:
                                   u * 1024 + c * 512 + 512], v_ps[:],
                            AF.Identity, scale=1.0 / 64.0)
                    nc.vector.tensor_copy(
                        v_aug[:, u * 1040: u * 1040 + 1040]
                        .rearrange("p (h dd) -> p h dd", dd=65)[:, :, 0:64],
                        v_full[:, u * 1024: u * 1024 + 1024]
                        .rearrange("p (h dd) -> p h dd", dd=64))
        qkv_cm.__exit__(None, None, None)

        # MoE weights prefetch on the vector ring (overlaps attention);
        # wp/x residual loads early on the sync ring so proj never waits
        moe_cm = tc.tile_pool(name="moe", bufs=1, side="right")
        moe = moe_cm.__enter__()
        w1_sb = moe.tile([128, 32768], fp8)
        w2_sb = moe.tile([128, 32768], fp8)
        for et in range(8):
            nc.gpsimd.dma_start(w1_sb[:, et * 4096: et * 4096 + 4096],
                                w1_d[:, et * 4096: et * 4096 + 4096])
        for ft8 in range(8):
            nc.gpsimd.dma_start(w2_sb[:, ft8 * 4096: ft8 * 4096 + 4096],
                                w2_d[:, ft8 * 4096: ft8 * 4096 + 4096])

        # warm the collective channel so the first real AllGather doesn't
        # pay rendezvous/ring-warmup latency on the critical path
        nc.gpsimd.collective_compute(
            "AllGather", mybir.AluOpType.bypass,
            replica_groups=[list(range(8))],
            ins=[warm_in.opt()], outs=[warm_out.opt()])

        # ============ phase 2: attention ============
        mask_sb = attn.tile([128, 1024], bf)
        for ss in range(4):
            nc.sync.dma_start(mask_sb[:, ss * 256: ss * 256 + 256],
                              masks_d[ss])

        proj_cm = tc.tile_pool(name="proj", bufs=1, side="right")
        projp = proj_cm.__enter__()
        catT_sb = projp.tile([128, 4096], bf)    # unnormalized heads^T
        catT8 = projp.tile([128, 4096], fp8)     # normalized, fp8 for proj
        wp_sb = projp.tile([128, 8192], fp8)
        nc.sync.dma_start(wp_sb[:], wp_d[:])
        x_sb = projp.tile([128, 4096], f32)
        nc.sync.dma_start(x_sb[:], xr_d[:])

        with tc.tile_pool(name="sc", bufs=4, side="left") as scp, \
             tc.tile_pool(name="ps_sc", bufs=2, space="PSUM") as ps_sc, \
             tc.tile_pool(name="ps_av", bufs=4, space="PSUM") as ps_av:
            # one N-restricted matmul per key-tile: keys (ph,tl) attend to
            # queries >= tl, so scores/exp/AV each run [128, (4-tl)*128].
            # PE emission is software-pipelined: scores run one key-tile
            # ahead of AV (covering exp latency), and each j's epilogue
            # (softmax-normalize via K=1 matmuls) is deferred into j+1.

            def epilogue(j, avps):
                # partition-shifting copies (0:64 -> 64:128, row 64 -> 0)
                # must run on ACT; lane-aligned ones go to DVE
                rcs = []
                for h01 in range(2):
                    if h01 == 0:
                        nc.vector.tensor_copy(
                            catT_sb[0:64, j * 512: j * 512 + 512],
                            avps[0][0:64, :])
                    else:
                        nc.scalar.copy(
                            catT_sb[64:128, j * 512: j * 512 + 512],
                            avps[1][0:64, :])
                    sm_f = scp.tile([1, 512], f32, tag="sm_f", bufs=4,
                                    name=f"smf{j}_{h01}")
                    nc.scalar.copy(sm_f[:], avps[h01][64:65, :])
                    rc_f = scp.tile([1, 512], f32, tag="rc_f", bufs=4,
                                    name=f"rcf{j}_{h01}")
                    nc.vector.reciprocal_approx_fast(rc_f[:], sm_f[:])
                    rc_h = scp.tile([1, 512], bf, tag="rc_h", bufs=4,
                                    name=f"rc{j}_{h01}")
                    nc.vector.tensor_copy(rc_h[:], rc_f[:])
                    rcs.append(rc_h)
                bc_ps = ps_sc.tile([128, 1024], f32, tag="sc",
                                   name=f"bc{j}")[:, 0:512]
                nc.tensor.matmul(bc_ps[0:64, :], ones_row[:, 0:64],
                                 rcs[0][:], start=True, stop=True)
                nc.tensor.matmul(bc_ps[64:128, :], ones_row[:, 0:64],
                                 rcs[1][:], start=True, stop=True)
                nc.vector.tensor_tensor(
                    catT_sb[:, j * 512: j * 512 + 512],
                    catT_sb[:, j * 512: j * 512 + 512], bc_ps[:],
                    op=ALU.mult)
                nc.vector.tensor_copy(catT8[:, j * 512: j * 512 + 512],
                                      catT_sb[:, j * 512: j * 512 + 512])

            # scores packed in (tl, tl+1) pairs -> one exp per pair; AV
            # runs one pair behind so exp latency is always covered
            pend = None
            for j in range(8):
                avps = [ps_av.tile([65, 512], f32, tag="av", name=f"av{j}_{k}")
                        for k in range(2)]
                for h01 in range(2):
                    po = 64 * h01
                    prev = None
                    first = True

                    def emit_av(pavs, start):
                        for pqo, pu, pexp, plo, pN in pavs:
                            nc.tensor.matmul(
                                avps[h01][:, pqo:512],
                                v_aug[:, pu * 1040 + (2 * j + h01) * 65:
                                      pu * 1040 + (2 * j + h01) * 65 + 65],
                                pexp[:, plo:plo + pN],
                                start=start, stop=False)
                            start = False

                    for ph in range(2):
                        for ta in (0, 2):
                            Na = (4 - ta) * 128
                            Nb = (4 - ta - 1) * 128
                            sc_ps = ps_sc.tile([128, 1024], f32, tag="sc",
                                               name=f"sc{j}_{h01}_{ph}_{ta}")
                            nc.tensor.matmul(
                                sc_ps[:, 0:Na],
                                kT_full[po:po + 64,
                                        j * 1024 + ph * 512 + ta * 128:
                                        j * 1024 + ph * 512 + ta * 128 + 128],
                                qT_sb[po:po + 64,
                                      j * 512 + ta * 128: j * 512 + 512],
                                start=True, stop=True)
                            nc.tensor.matmul(
                                sc_ps[:, Na:Na + Nb],
                                kT_full[po:po + 64,
                                        j * 1024 + ph * 512 + ta * 128 + 128:
                                        j * 1024 + ph * 512 + ta * 128 + 256],
                                qT_sb[po:po + 64,
                                      j * 512 + ta * 128 + 128: j * 512 + 512],
                                start=True, stop=True)
                            expT = scp.tile([128, 1024], bf, tag="expT",
                                            name=f"ex{j}_{h01}_{ph}_{ta}")
                            nc.scalar.activation(
                                expT[:, 0:Na], sc_ps[:, 0:Na], AF.Exp)
                            nc.scalar.activation(
                                expT[:, Na:Na + Nb], sc_ps[:, Na:Na + Nb],
                                AF.Exp)
                            nc.vector.tensor_tensor(
                                expT[:, 0:128], expT[:, 0:128],
                                mask_sb[:, ta * 256 + ph * 128:
                                        ta * 256 + ph * 128 + 128],
                                op=ALU.mult)
                            nc.vector.tensor_tensor(
                                expT[:, Na:Na + 128], expT[:, Na:Na + 128],
                                mask_sb[:, (ta + 1) * 256 + ph * 128:
                                        (ta + 1) * 256 + ph * 128 + 128],
                                op=ALU.mult)
                            if prev is not None:
                                emit_av(prev, first)
                                first = False
                            if pend is not None and h01 == 0 and \
                                    ph == 1 and ta == 0:
                                epilogue(*pend)
                                pend = None
                            u = ph * 4 + ta
                            prev = [(ta * 128, u, expT, 0, Na),
                                    ((ta + 1) * 128, u + 1, expT, Na, Nb)]
                    emit_av(prev[:1], first)
                    pqo, pu, pexp, plo, pN = prev[1]
                    nc.tensor.matmul(
                        avps[h01][:, pqo:512],
                        v_aug[:, pu * 1040 + (2 * j + h01) * 65:
                              pu * 1040 + (2 * j + h01) * 65 + 65],
                        pexp[:, plo:plo + pN],
                        start=False, stop=True)
                pend = (j, avps)
                if j == 6:
                    # re-warm the collective channel close to the real AGs
                    nc.gpsimd.collective_compute(
                        "AllGather", mybir.AluOpType.bypass,
                        replica_groups=[list(range(8))],
                        ins=[warm2_in.opt()], outs=[warm2_out.opt()])
            epilogue(*pend)
        attn_cm.__exit__(None, None, None)

        h_sb = projp.tile([128, 4096], bf)
        hT_stage = projp.tile([128, 4096], fp8)
        cat_v = catT8[:].rearrange("p (j t) -> p j t", j=8)
        wp_v = wp_sb[:].rearrange("p (j e) -> p j e", j=8)

        with tc.tile_pool(name="prw", bufs=2, side="left") as prp, \
             tc.tile_pool(name="ps_pr", bufs=4, space="PSUM") as ps_pr:
            for tt in range(4):
                y_sb = prp.tile([128, 1024], f32, tag="y")
                for ec in range(2):
                    ao_ps = ps_pr.tile([128, 512], f32, tag="ao")
                    for p in range(4):
                        nc.tensor.matmul(
                            ao_ps[:],
                            cat_v[:, 2 * p: 2 * p + 2,
                                  tt * 128: tt * 128 + 128],
                            wp_v[:, 2 * p: 2 * p + 2,
                                 ec * 512: ec * 512 + 512],
                            start=(p == 0), stop=False, perf_mode=DR)
                    nc.tensor.matmul(
                        ao_ps[:], ones_row[:, 0:128],
                        bp_sb[:, ec * 512: ec * 512 + 512],
                        start=False, stop=True)
                    nc.vector.tensor_scalar(
                        y_sb[:, ec * 512: ec * 512 + 512], ao_ps[:],
                        1.0 / 64.0, None, op0=ALU.mult)
                    nc.vector.tensor_tensor(
                        y_sb[:, ec * 512: ec * 512 + 512],
                        y_sb[:, ec * 512: ec * 512 + 512],
                        x_sb[:, tt * 1024 + ec * 512: tt * 1024 + ec * 512 + 512],
                        op=ALU.add)
                # LN1 stats
                mean = prp.tile([128, 1], f32, tag="mean")
                nc.vector.reduce_sum(mean[:], y_sb[:], axis=mybir.AxisListType.X)
                nc.vector.tensor_scalar_mul(mean[:], mean[:], 1.0 / 1024.0)
                sq = prp.tile([128, 1024], bf, tag="sq")
                sqs = prp.tile([128, 1], f32, tag="sqs")
                nc.scalar.activation(sq[:], y_sb[:], AF.Square,
                                     accum_out=sqs[:])
                m2 = prp.tile([128, 1], f32, tag="m2")
                nc.scalar.activation(m2[:], mean[:], AF.Square)
                var = prp.tile([128, 1], f32, tag="var")
                nc.vector.tensor_scalar(var[:], sqs[:], 1.0 / 1024.0, EPS,
                                        op0=ALU.mult, op1=ALU.add)
                nc.vector.tensor_tensor(var[:], var[:], m2[:], op=ALU.subtract)
                std = prp.tile([128, 1], f32, tag="std")
                nc.scalar.activation(std[:], var[:], AF.Sqrt)
                rstd = prp.tile([128, 1], f32, tag="rstd")
                nc.vector.reciprocal(rstd[:], std[:])
                nc.vector.tensor_scalar(
                    h_sb[:, tt * 1024: tt * 1024 + 1024], y_sb[:],
                    mean[:], rstd[:], op0=ALU.subtract, op1=ALU.mult)
                # transpose h tile -> hT
                for et in range(8):
                    tp = ps_pr.tile([128, 128], bf, tag="tp")
                    nc.tensor.transpose(
                        tp[:], h_sb[:, tt * 1024 + et * 128:
                                    tt * 1024 + et * 128 + 128], ident_sb[:])
                    nc.scalar.copy(
                        hT_stage[:, et * 512 + tt * 128:
                                 et * 512 + tt * 128 + 128], tp[:])
                hT_v = hT_stage[:].rearrange("p (e c) -> p e c", e=8)
                if tt == 0:
                    nc.gpsimd.dma_start(
                        hag_inA1.rearrange("e p c -> p e c"),
                        hT_v[:, :, 0:128])
                    nc.gpsimd.collective_compute(
                        "AllGather", mybir.AluOpType.bypass,
                        replica_groups=[list(range(8))],
                        ins=[hag_inA1.opt()], outs=[hag_outA1.opt()])
                if tt == 1:
                    nc.gpsimd.dma_start(
                        hag_inA2.rearrange("e p c -> p e c"),
                        hT_v[:, :, 128:256])
                    nc.gpsimd.collective_compute(
                        "AllGather", mybir.AluOpType.bypass,
                        replica_groups=[list(range(8))],
                        ins=[hag_inA2.opt()], outs=[hag_outA2.opt()])
                if tt == 3:
                    nc.gpsimd.dma_start(
                        hag_inB.rearrange("e p c -> p e c"),
                        hT_v[:, :, 256:512])
                    nc.gpsimd.collective_compute(
                        "AllGather", mybir.AluOpType.bypass,
                        replica_groups=[list(range(8))],
                        ins=[hag_inB.opt()], outs=[hag_outB.opt()])
            # (b) PE warm-keepers over the h-AG wait
            for wi in range(16):
                wm_ps = ps_pr.tile([128, 512], f32, tag="ao", name=f"wm{wi}")
                nc.tensor.matmul(wm_ps[:], catT8[:, 0:128],
                                 wp_sb[:, 0:512], start=True, stop=True)
        proj_cm.__exit__(None, None, None)

        # ============ phase 3: MoE (expert r over all tokens) ============
        # fin (residual+LN2) work is interleaved per completed RS group so
        # only the last group's LN2 sits on the tail
        with tc.tile_pool(name="fin", bufs=2, side="left") as fin, \
             tc.tile_pool(name="mchunk", bufs=2, side="left") as mck, \
             tc.tile_pool(name="ps_md", bufs=2, space="PSUM") as ps_md, \
             tc.tile_pool(name="ps_eo", bufs=3, space="PSUM") as ps_eo:
            x2_sb = fin.tile([64, 8192], f32, bufs=1)
            nc.sync.dma_start(x2_sb[:], xr2_d[:])
            g2_sb = fin.tile([128, 1024], f32, bufs=1)
            nc.sync.dma_start(g2_sb[:], g2_d[:])
            bl2_sb = fin.tile([128, 1024], f32, bufs=1)
            nc.sync.dma_start(bl2_sb[:], bl2_d[:])

            def do_fin(g):
                y2b = fin.tile([64, 1024], bf, tag="y2b", name=f"y2b{g}")
                nc.sync.dma_start(y2b[:], rs_outs[g][:])
                y2 = fin.tile([64, 1024], f32, tag="y2", name=f"y2{g}")
                nc.vector.tensor_tensor(
                    y2[:], y2b[:], x2_sb[:, g * 1024: g * 1024 + 1024],
                    op=ALU.add)
                mean = fin.tile([64, 1], f32, tag="mean2")
                nc.vector.reduce_sum(mean[:], y2[:], axis=mybir.AxisListType.X)
                nc.vector.tensor_scalar_mul(mean[:], mean[:], 1.0 / 1024.0)
                sq = fin.tile([64, 1024], f32, tag="sq2")
                sqs = fin.tile([64, 1], f32, tag="sqs2")
                nc.scalar.activation(sq[:], y2[:], AF.Square, accum_out=sqs[:])
                m2 = fin.tile([64, 1], f32, tag="m22")
                nc.scalar.activation(m2[:], mean[:], AF.Square)
                var = fin.tile([64, 1], f32, tag="var2")
                nc.vector.tensor_scalar(var[:], sqs[:], 1.0 / 1024.0, EPS,
                                        op0=ALU.mult, op1=ALU.add)
                nc.vector.tensor_tensor(var[:], var[:], m2[:],
                                        op=ALU.subtract)
                std = fin.tile([64, 1], f32, tag="std2")
                nc.scalar.activation(std[:], var[:], AF.Sqrt)
                rstd = fin.tile([64, 1], f32, tag="rstd2")
                nc.vector.reciprocal(rstd[:], std[:])
                on = fin.tile([64, 1024], f32, tag="on", name=f"on{g}")
                nc.vector.tensor_scalar(on[:], y2[:], mean[:], rstd[:],
                                        op0=ALU.subtract, op1=ALU.mult)
                nc.vector.tensor_tensor(on[:], on[:], g2_sb[0:64, :],
                                        op=ALU.mult)
                nc.vector.tensor_tensor(on[:], on[:], bl2_sb[0:64, :],
                                        op=ALU.add)
                nc.sync.dma_start(out_d[g], on[:])
            DR = mybir.MatmulPerfMode.DoubleRow
            # 512-token chunks: ck<4 pairs two cores' A-halves (available
            # right after the early AllGathers), ck>=4 pairs their B-halves
            for ck in range(8):
                hT_c = mck.tile([128, 4096], fp8, tag="hT_c")
                hc_v = hT_c[:].rearrange("p (e c) -> p e c", e=8)
                if ck < 4:
                    s0, s1 = 2 * ck, 2 * ck + 1
                    nc.gpsimd.dma_start(
                        hc_v[:, :, 0:128],
                        hag_outA1[s0].rearrange("e p c -> p e c"))
                    nc.gpsimd.dma_start(
                        hc_v[:, :, 128:256],
                        hag_outA2[s0].rearrange("e p c -> p e c"))
                    nc.gpsimd.dma_start(
                        hc_v[:, :, 256:384],
                        hag_outA1[s1].rearrange("e p c -> p e c"))
                    nc.gpsimd.dma_start(
                        hc_v[:, :, 384:512],
                        hag_outA2[s1].rearrange("e p c -> p e c"))
                else:
                    s0, s1 = 2 * (ck - 4), 2 * (ck - 4) + 1
                    nc.gpsimd.dma_start(
                        hc_v[:, :, 0:256],
                        hag_outB[s0].rearrange("e p c -> p e c"))
                    nc.gpsimd.dma_start(
                        hc_v[:, :, 256:512],
                        hag_outB[s1].rearrange("e p c -> p e c"))
                midT = mck.tile([128, 16384], fp8, tag="midT", bufs=2)
                for ft in range(32):
                    md_ps = ps_md.tile([128, 512], f32, tag="md")
                    for p in range(4):
                        nc.tensor.matmul(
                            md_ps[:],
                            w1_sb[:, p * 8192 + ft * 256:
                                  p * 8192 + ft * 256 + 256]
                            .rearrange("q (k2 m) -> q k2 m", k2=2),
                            hT_c[:, p * 1024: p * 1024 + 1024]
                            .rearrange("q (k2 t) -> q k2 t", k2=2),
                            start=(p == 0), stop=(p == 3), perf_mode=DR)
                    nc.scalar.activation(
                        midT[:, ft * 512: ft * 512 + 512], md_ps[:],
                        AF.Relu, bias=b1_sb[:, ft: ft + 1])
                # router after mid: pcol only needed at the eo scale step,
                # and this keeps the PE on mid work right after the hT DMA
                pcol = mck.tile([128, 4], f32, tag="pcol")
                for th in range(4):
                    lg_ps = ps_eo.tile([128, 8], f32, tag="lg")
                    for p in range(4):
                        nc.tensor.matmul(
                            lg_ps[:],
                            hT_c[:, p * 1024: p * 1024 + 1024]
                            .rearrange("q (k2 t) -> q k2 t", k2=2)
                            [:, :, th * 128: th * 128 + 128],
                            wr_sb[:, p * 16: p * 16 + 16]
                            .rearrange("q (k2 n) -> q k2 n", k2=2),
                            start=(p == 0), stop=False, perf_mode=DR)
                    nc.tensor.matmul(lg_ps[:], ones_row[:, 0:128], br_sb[:],
                                     start=False, stop=True)
                    pe = mck.tile([128, 8], f32, tag="pe")
                    ps = mck.tile([128, 1], f32, tag="ps")
                    nc.scalar.activation(pe[:], lg_ps[:], AF.Exp,
                                         scale=1.0 / 64.0, accum_out=ps[:])
                    ps2 = mck.tile([128, 1], f32, tag="ps2")
                    nc.vector.tensor_scalar_mul(ps2[:], ps[:], 1024.0)
                    pr = mck.tile([128, 1], f32, tag="pr")
                    nc.vector.reciprocal(pr[:], ps2[:])
                    nc.vector.tensor_tensor(pcol[:, th:th + 1], pe[:, 0:1],
                                            pr[:], op=ALU.mult)
                eo_sb = mck.tile([128, 4096], bf, tag="eo", bufs=2)
                for th in range(4):
                    for ec in range(2):
                        eo_ps = ps_eo.tile([128, 512], f32, tag="eo_ps")
                        for qq2 in range(16):
                            nc.tensor.matmul(
                                eo_ps[:],
                                midT[:, qq2 * 1024: qq2 * 1024 + 1024]
                                .rearrange("q (k2 t) -> q k2 t", k2=2)
                                [:, :, th * 128: th * 128 + 128],
                                w2_sb[:, qq2 * 2048: qq2 * 2048 + 2048]
                                .rearrange("q (k2 e) -> q k2 e", k2=2)
                                [:, :, ec * 512: ec * 512 + 512],
                                start=(qq2 == 0), stop=False, perf_mode=DR)
                        nc.tensor.matmul(
                            eo_ps[:], ones_row[:, 0:128],
                            b2_sb[:, ec * 512: ec * 512 + 512],
                            start=False, stop=True)
                        if ec == 0:
                            nc.scalar.activation(
                                eo_sb[:, th * 1024 + ec * 512:
                                      th * 1024 + ec * 512 + 512],
                                eo_ps[:], AF.Identity,
                                scale=pcol[:, th: th + 1])
                        else:
                            nc.vector.tensor_scalar_mul(
                                eo_sb[:, th * 1024 + ec * 512:
                                      th * 1024 + ec * 512 + 512],
                                eo_ps[:], pcol[:, th: th + 1])
                for th in range(4):
                    nc.sync.dma_start(
                        rs_ins[ck][th * 128: th * 128 + 128, :],
                        eo_sb[:, th * 1024: th * 1024 + 1024])
                nc.gpsimd.collective_compute(
                    "ReduceScatter", mybir.AluOpType.add,
                    replica_groups=[list(range(8))],
                    ins=[rs_ins[ck].opt()], outs=[rs_outs[ck].opt()])
                if ck >= 2:
                    do_fin(ck - 2)
            do_fin(6)
            do_fin(7)
        moe_cm.__exit__(None, None, None)

        cpool_cm.__exit__(None, None, None)
    nc.compile()
    return nc


def _prep_inputs(inputs):
    f = np.float32
    x = np.asarray(inputs["x"], f)
    wq, bq = np.asarray(inputs["wq"], f), np.asarray(inputs["bq"], f)
    wk, bk = np.asarray(inputs["wk"], f), np.asarray(inputs["bk"], f)
    wv, bv = np.asarray(inputs["wv"], f), np.asarray(inputs["bv"], f)
    wp, bp = np.asarray(inputs["wp"], f), np.asarray(inputs["bp"], f)
    ln1_g, ln1_b = np.asarray(inputs["ln1_g"], f), np.asarray(inputs["ln1_b"], f)
    ln2_g, ln2_b = np.asarray(inputs["ln2_g"], f), np.asarray(inputs["ln2_b"], f)
    wr, br = np.asarray(inputs["wr"], f), np.asarray(inputs["br"], f)
    w1, b1 = np.asarray(inputs["w1"], f), np.asarray(inputs["b1"], f)
    w2, b2 = np.asarray(inputs["w2"], f), np.asarray(inputs["b2"], f)

    def etile(a):  # [E, M] -> [128, 8*M]
        M = a.shape[1]
        return np.ascontiguousarray(
            a.reshape(8, 128, M).transpose(1, 0, 2).reshape(128, 8 * M))

    wq_f = wq.transpose(1, 0, 2).reshape(E, E)   # [e, h*64+d]
    wk_f = wk.transpose(1, 0, 2).reshape(E, E)
    wv_f = wv.transpose(1, 0, 2).reshape(E, E)
    # sect-major [k|v|q], each section e-tiled: contiguous section DMAs
    wqkv_t = np.concatenate(
        [etile(64.0 * wk_f), etile(64.0 * wv_f), etile(64.0 * wq_f)],
        axis=1).astype(F8)                                   # [128, 3*8192]
    bqk = 64.0 * np.concatenate(
        [bq.reshape(-1).reshape(8, 128).T,
         bk.reshape(-1).reshape(8, 128).T], axis=1).astype(f)
    wp_t = etile(64.0 * wp).astype(F8)                       # [128, 8*1024]
    w1e = (ln1_g[:, None] * w1).astype(f)                    # [n,E,FF]
    b1e = b1 + ln1_b @ w1                                    # [n,FF]
    wre = (ln1_g[:, None] * wr).astype(f)                    # [E,8]
    bre = br + ln1_b @ wr                                    # [8]
    ident = np.eye(128, dtype=BF16)

    in_maps = []
    for r in range(NCORE):
        b, p = r // 2, r % 2
        # interleaved token assignment: local s_loc <-> orig row 2*s_loc + p
        xs = np.ascontiguousarray(x[b, p::2, :])             # [512, E]
        xpart = np.ascontiguousarray(x[b, 1 - p::2, :])      # partner tokens
        xT_t = np.concatenate(
            [etile(np.ascontiguousarray(xs.T)),
             etile(np.ascontiguousarray(xpart.T))], axis=1).astype(F8)
        xr_t = np.ascontiguousarray(
            xs.reshape(4, 128, 1024).transpose(1, 0, 2).reshape(128, 4096), f)
        # final-phase x rows: group k = batch k%4, half A (k<4) or B;
        # rows [src-core-2b tokens | src-core-2b+1 tokens], this core's
        # 64-row slice starts at 64r
        R = 64 * r + np.arange(64)
        xr2_cols = []
        for k in range(8):
            b = k % 4
            local = R % 256 + (0 if k < 4 else 256)
            par = R // 256
            xr2_cols.append(x[b, 2 * local + par, :])
        xr2_t = np.ascontiguousarray(np.concatenate(xr2_cols, axis=1), f)
        # diagonal causal masks: half0 = own parity keys, half1 = partner
        masks = np.zeros((4, 128, 256), BF16)
        ti = np.arange(128)
        sj = np.arange(128)
        own = (ti[:, None] <= sj[None, :])
        part = (ti[:, None] <= sj[None, :]) if p == 1 else                (ti[:, None] < sj[None, :])
        for ss in range(4):
            masks[ss][:, 0:128] = own.astype(BF16)
            masks[ss][:, 128:256] = part.astype(BF16)
        perm = [r] + [i for i in range(NEXP) if i != r]
        # fp8 DoubleRow layouts; w1 x16, w2/wr x64 to clear e4m3 subnormals
        wr_p = etile(64.0 * wre[:, perm]).astype(F8)         # [128, 8*8]
        br_p = (64.0 * bre[perm]).reshape(1, 8).astype(BF16)
        w1_t = np.ascontiguousarray(
            (16.0 * w1e[r]).reshape(4, 2, 128, 32, 128)
            .transpose(2, 0, 3, 1, 4).reshape(128, 32768)).astype(F8)
        b1_t = np.ascontiguousarray(
            16.0 * b1e[r].reshape(32, 128).T, f)             # [128, 32]
        w2_t = np.ascontiguousarray(
            (64.0 * w2[r]).reshape(16, 2, 128, 1024)
            .transpose(2, 0, 1, 3).reshape(128, 32768)).astype(F8)
        in_maps.append({
            "xT": xT_t, "xr": xr_t, "xr2": xr2_t, "wqkv": wqkv_t, "bqk": bqk,
            "bv": (64.0 * bv).reshape(1, E).astype(BF16),
            "wp": wp_t, "bp": (64.0 * bp).reshape(1, E).astype(BF16),
            "masks": masks, "ident": ident,
            "w1": w1_t, "b1": b1_t, "w2": w2_t,
            "b2": (1024.0 * b2[r]).reshape(1, E).astype(BF16),
            "wr": wr_p, "br": br_p,
            "g2": np.broadcast_to(ln2_g, (128, E)).astype(f).copy(),
            "bl2": np.broadcast_to(ln2_b, (128, E)).astype(f).copy(),
        })
    return in_maps


def kernel(**inputs):
    from concourse import bass_utils
    if "nc" not in _CACHE:
        _CACHE["nc"] = _build_program()
    nc = _CACHE["nc"]
    in_maps = _prep_inputs(inputs)
    res = bass_utils.run_bass_kernel_spmd(
        nc, in_maps, core_ids=list(range(NCORE)))
    # group k = batch k%4, A-halves (k<4) or B-halves of the two src
    # cores; core r holds rows [64r, 64r+64) of each group
    full = np.empty((B, S, E), np.float32)
    for r in range(NCORE):
        o = res.results[r]["out"]                            # [8, 64, 1024]
        R = 64 * r + np.arange(64)
        for k in range(NCORE):
            b = k % 4
            local = R % 256 + (0 if k < 4 else 256)
            par = R // 256
            full[b, 2 * local + par, :] = o[k]
    return full

